# revision 1
# baseline (speedup 1.0000x reference)
"""Bass/Tile TRN2 kernel for nn_BiasedMultiheadAttention (B=2,T=2048,D=1024,H=16,DM=256).

Sharding: 8 cores = batch(2) x head-group(4).  Each core computes 4 heads of one
batch element plus the (replicated) Gm bias linear, and emits an unnormalized
partial of the output projection; the host sums the 4 partials per batch and
adds bo.

Per-core pipeline (projection/QK matmuls in float32r; bias+attnV in bf16):
  A) projections    QT=(Wq_c @ [hx;Hf]^T)/8, KT, V (transposed layouts)
  B) attention      scoresT = K_h Q_h^T; bias applied via exp(a+b)=exp(a)exp(b):
                    biasE=exp(GmWg^T+bg) [bf16], ex=exp(qk)*biasE [bf16]; no
                    max-subtraction (|scores| <= ~8).  attnV uses
                    lhsT=[V_h|ones]/[ones|V_h] so softmax denominators land on
                    the 64 PSUM partitions the head's data doesn't use;
                    normalize with DVE reciprocal after a cross-partition DMA.
  C) out projection partial = outT^T @ Wo[:,C]^T, interleaved after each qc.

All three phases share one PSUM pool with four 2-bank tags (qk0/qk1/bias/out)
so no pool alloc/release barrier ever serializes phase transitions.
"""

import numpy as np
import ml_dtypes

import concourse.bass as bass
from concourse import bacc
import concourse.mybir as mybir
from concourse.tile import TileContext
from concourse.bass_utils import run_bass_kernel_spmd

B, T, D, H, DM = 2, 2048, 1024, 16, 256
P = 128
F32 = mybir.dt.float32
BF16 = mybir.dt.bfloat16
F32R = mybir.dt.float32r
AF = mybir.ActivationFunctionType
OP = mybir.AluOpType


def _pm(a, dt=np.float32):
    """(R, C) row-major -> partition-major (128, R//128, C), contiguous."""
    a = np.ascontiguousarray(a, dtype=np.float32)
    r, c = a.shape
    return np.ascontiguousarray(a.reshape(r // P, P, c).transpose(1, 0, 2).astype(dt))


def build_nc():
    nc = bacc.Bacc("TRN2", target_bir_lowering=False, debug=False)

    def inp(name, shape, dt=F32):
        return nc.dram_tensor(name, list(shape), dt, kind="ExternalInput")

    dXa = inp("XTa", (P, 8, 2048), F32R)
    dXb = inp("XTb", (P, 8, 2048), F32R)
    dWqa = inp("WqaT", (P, 8, 256), F32R)
    dWqb = inp("WqbT", (P, 8, 256), F32R)
    dWka = inp("WkaT", (P, 8, 256), F32R)
    dWkb = inp("WkbT", (P, 8, 256), F32R)
    dWv = inp("WvT", (P, 8, 256), F32R)
    dWo = inp("WoT", (P, 2, 1024), BF16)
    dGm = inp("GmT", (P, 2, 2048), BF16)
    dWg = inp("WgT", (P, 2, 2048), BF16)
    dbq = inp("bq2", (P, 2))
    dbk = inp("bk2", (P, 2))
    dbv = inp("bvb", (P, 256))
    dbg = inp("bgRow", (1, 2048), BF16)
    dons = inp("onesRow", (1, 512), BF16)
    dout = nc.dram_tensor("outp", [P, 16, 1024], F32, kind="ExternalOutput")

    with TileContext(nc) as tc:
        with tc.tile_pool(name="sb", bufs=1) as sb, \
             tc.tile_pool(name="ps", bufs=1, space="PSUM") as ps:
            QT = sb.tile([P, 2, 2048], F32R, tag="QT")
            KT = sb.tile([P, 2, 2048], F32R, tag="KT")
            Vt = sb.tile([P, 16, 512], BF16, tag="Vt")  # per tb: [V|1]/[1|V] x2
            oT = sb.tile([P, 2, 2048], BF16, tag="oT")
            woT = sb.tile([P, 2, 1024], BF16, tag="woT")
            gmT = sb.tile([P, 2, 2048], BF16, tag="gmT")
            wgT = sb.tile([P, 2, 2048], BF16, tag="wgT")
            bq_s = sb.tile([P, 2], F32, tag="bq")
            bk_s = sb.tile([P, 2], F32, tag="bk")
            bgR = sb.tile([1, 2048], BF16, tag="bgR")
            ones1 = sb.tile([1, 512], BF16, tag="ones1")

            nc.sync.dma_start(bq_s[:], dbq[:])
            nc.sync.dma_start(bk_s[:], dbk[:])
            nc.sync.dma_start(bgR[:], dbg[:])
            nc.sync.dma_start(ones1[:], dons[:])
            nc.vector.memset(Vt[:], 1.0)

            # ---------------- Phase A: projections ----------------
            with tc.tile_pool(name="pv", bufs=1) as pv:
                for half in range(2):
                    dX = dXa if half == 0 else dXb
                    xio = [sb.tile([P, 2048], F32R, tag=f"x{io}", name=f"x{io}")
                           for io in range(8)]
                    nc.sync.dma_start(xio[0][:], dX[:, 0, :])
                    wq_s = sb.tile([P, 8, 256], F32R, tag="wq", name="wq_s")
                    wk_s = sb.tile([P, 8, 256], F32R, tag="wk", name="wk_s")
                    if half == 0:
                        wv_s = pv.tile([P, 8, 256], F32R, tag="wv")
                        bvb_s = pv.tile([P, 256], F32, tag="bvb")
                        nc.sync.dma_start(wv_s[:], dWv[:])
                        nc.sync.dma_start(bvb_s[:], dbv[:])
                    nc.sync.dma_start(wq_s[:], dWqa[:] if half == 0 else dWqb[:])
                    nc.sync.dma_start(wk_s[:], dWka[:] if half == 0 else dWkb[:])
                    for io in range(1, 8):
                        nc.sync.dma_start(xio[io][:], dX[:, io, :])
                    # early loads for phase B: gm/wg right after the first x
                    # half so qc0's bias pipeline can run during the second
                    # half's x load
                    if half == 0:
                        nc.sync.dma_start(gmT[:], dGm[:])
                        nc.sync.dma_start(wgT[:], dWg[:])
                    else:
                        nc.sync.dma_start(woT[:], dWo[:])
                    if half == 0:
                        # V: out (t=128, jc=256), contraction over i (hx only)
                        for tb in range(16):
                            vt = ps.tile([P, 2, 512], F32,
                                         tag=("bias" if tb % 2 else "out"), name="vt")
                            vps = vt[:, 0, 0:256]
                            for io in range(8):
                                nc.tensor.matmul(
                                    vps,
                                    lhsT=xio[io][:, tb * P:(tb + 1) * P],
                                    rhs=wv_s[:, io, :],
                                    start=(io == 0), stop=(io == 7),
                                )
                            # write data columns of Vt (+bv); ones from memset
                            nc.vector.tensor_tensor(
                                Vt[:, tb].rearrange("p (a u v) -> p a u v", a=2, u=4)[:, :, 0:4:3, :],
                                vps.rearrange("p (a u v) -> p a u v", a=2, u=2),
                                bvb_s.rearrange("p (a u v) -> p a u v", a=2, u=2),
                                OP.add,
                            )
                    # Q/K chain order: phase B(qc0) needs Q(qc0) then K over
                    # ALL qc-chunks (keys span the full sequence), so in half 2
                    # emit Q(qc0), K(qc0..3), then the remaining Q chunks.
                    def emit_chain(wt, dst, bias_s, qc, wi):
                        qs = slice(qc * 512, (qc + 1) * 512)
                        t = ps.tile([P, 2, 512], F32, tag=f"qk{wi}", name="t")
                        for jb in range(2):
                            for io in range(8):
                                nc.tensor.matmul(
                                    t[:, jb, :],
                                    lhsT=wt[:, io, jb * P:(jb + 1) * P],
                                    rhs=xio[io][:, qs],
                                    start=(io == 0), stop=(io == 7),
                                )
                        if half == 0:
                            for jb in range(2):
                                nc.vector.tensor_scalar_add(
                                    dst[:, jb, qs], t[:, jb, :], bias_s[:, jb:jb + 1])
                        else:
                            nc.vector.tensor_tensor(
                                dst[:, :, qs], t[:], dst[:, :, qs], OP.add)

                    if half == 0:
                        for qc in range(4):
                            emit_chain(wq_s, QT, bq_s, qc, 0)
                            emit_chain(wk_s, KT, bk_s, qc, 1)
                    else:
                        emit_chain(wq_s, QT, bq_s, 0, 0)
                        for qc in range(4):
                            emit_chain(wk_s, KT, bk_s, qc, 1)
                        for qc in range(1, 4):
                            emit_chain(wq_s, QT, bq_s, qc, 0)

            # ---------------- Phase B + C ----------------
            with tc.tile_pool(name="pl", bufs=1) as pl, \
                 tc.tile_pool(name="plr", bufs=3) as plr:
                def emit_c(qct, j, tag="bias"):
                    qb = 4 * qct + j
                    po = ps.tile([P, 2, 512], F32, tag=tag, name="po")
                    for jh in range(2):
                        for ch in range(2):
                            nc.tensor.matmul(
                                po[:, jh, :],
                                lhsT=oT[:, ch, qb * P:(qb + 1) * P],
                                rhs=woT[:, ch, jh * 512:(jh + 1) * 512],
                                start=(ch == 0), stop=(ch == 1),
                            )
                    osb = plr.tile([P, 2, 512], F32, tag="osb", name="osb")
                    nc.scalar.activation(osb[:], po[:], AF.Copy)
                    nc.sync.dma_start(dout[:, qb, :], osb.rearrange("p a b -> p (a b)"))

                def emit_bias(biasE, qct, j):
                    tqs = slice(qct * 512, (qct + 1) * 512)
                    bps = ps.tile([P, 2, 512], F32, tag="bias", name="bps")
                    for tbj in range(2):
                        tb = 2 * j + tbj
                        for ch in range(2):
                            nc.tensor.matmul(
                                bps[:, tbj, :],
                                lhsT=wgT[:, ch, tb * P:(tb + 1) * P],
                                rhs=gmT[:, ch, tqs],
                                start=(ch == 0), stop=False,
                            )
                        nc.tensor.matmul(  # += bg[t'] (rank-1, K=1)
                            bps[:, tbj, :],
                            lhsT=bgR[0:1, tb * P:(tb + 1) * P],
                            rhs=ones1[0:1, :],
                            start=False, stop=True,
                        )
                    nc.scalar.activation(biasE[:, 2 * j:2 * j + 2, :], bps[:], AF.Exp)

                for qc in range(4):
                    qs = slice(qc * 512, (qc + 1) * 512)
                    biasE = pl.tile([P, 16, 512], BF16, tag="biasE", name="biasE")
                    if qc == 0:
                        # fill the B-start window (x half 2 still loading)
                        for j in range(8):
                            emit_bias(biasE, qc, j)
                    for p in range(2):
                        op = ps.tile([P, 2, 512], F32, tag="out", name="op")
                        if p == 0 and qc > 0:
                            emit_bias(biasE, qc, 0)
                        for tb in range(16):
                            qk = ps.tile([P, 2, 512], F32, tag=f"qk{tb % 2}",
                                         name=f"qk{tb % 2}")
                            for hh in range(2):
                                dd = slice(hh * 64, (hh + 1) * 64)
                                nc.tensor.matmul(
                                    qk[:, hh, :],
                                    lhsT=KT[dd, p, tb * P:(tb + 1) * P],
                                    rhs=QT[dd, p, qs],
                                    start=True, stop=True,
                                )
                            if qc > 0 and p == 0 and tb % 2 == 1 and tb < 15:
                                emit_bias(biasE, qc, (tb + 1) // 2)
                            ex1 = plr.tile([P, 2, 512], BF16, tag="ex1", name="ex1")
                            nc.scalar.activation(ex1[:], qk[:], AF.Exp)
                            ex = plr.tile([P, 2, 512], BF16, tag="ex", name="ex")
                            for hh in range(2):
                                nc.vector.tensor_tensor(
                                    ex[:, hh, :], ex1[:, hh, :],
                                    biasE[:, tb, :], OP.mult)
                            if p == 1 and tb % 4 == 0 and qc > 0:
                                emit_c(qc - 1, tb // 4)
                            for hh in range(2):
                                h = 2 * p + hh
                                nc.tensor.matmul(
                                    op[:, hh, :],
                                    lhsT=Vt[:, tb, h * P:(h + 1) * P],
                                    rhs=ex[:, hh, :],
                                    start=(tb == 0), stop=(tb == 15),
                                )
                        # normalize: hh=0 data on parts 0:64 (denom on 64:128);
                        # hh=1 mirrored.  Engines can't cross partitions ->
                        # bounce denoms through SBUF with a gpsimd DMA.
                        t0 = pl.tile([P, 512], F32, tag="t0", name="t0")
                        rB = pl.tile([P, 512], F32, tag="rB", name="rB")
                        rC = pl.tile([P, 512], F32, tag="rC", name="rC")
                        nc.vector.tensor_copy(t0[64:128, :], op[:, 0, :][64:128, :])
                        nc.vector.tensor_copy(t0[0:64, :], op[:, 1, :][0:64, :])
                        nc.gpsimd.dma_start(rB[0:64, :], t0[64:128, :])
                        nc.gpsimd.dma_start(rB[64:128, :], t0[0:64, :])
                        nc.vector.reciprocal(rC[:], rB[:])
                        nc.vector.tensor_tensor(
                            oT[0:64, p, qs], op[:, 0, :][0:64, :], rC[0:64, :], OP.mult)
                        nc.vector.tensor_tensor(
                            oT[64:128, p, qs], op[:, 1, :][64:128, :], rC[64:128, :], OP.mult)

                # phase C for the last qc: all four psum tags are free, so
                # the four output chains run concurrently
                for j, tag in enumerate(("bias", "qk0", "qk1", "out")):
                    emit_c(3, j, tag)

    nc.compile()
    return nc


def _prep_core_inputs(inputs, core):
    b, g = core // 4, core % 4
    C = slice(g * 256, (g + 1) * 256)
    Hx = np.asarray(inputs["Hx"], np.float32)
    Hf = np.asarray(inputs["Hf"], np.float32)
    Gm = np.asarray(inputs["Gm"], np.float32)
    Wg = np.asarray(inputs["Wg"], np.float32)
    bg = np.asarray(inputs["bg"], np.float32)
    Wq = np.asarray(inputs["Wq"], np.float32)
    bq = np.asarray(inputs["bq"], np.float32)
    Wk = np.asarray(inputs["Wk"], np.float32)
    bk = np.asarray(inputs["bk"], np.float32)
    Wv = np.asarray(inputs["Wv"], np.float32)
    bv = np.asarray(inputs["bv"], np.float32)
    Wo = np.asarray(inputs["Wo"], np.float32)

    bf = ml_dtypes.bfloat16
    s = 1.0 / 8.0  # 1/sqrt(DK) folded into Q
    return {
        "XTa": _pm(Hx[b, :, :, 0]),
        "XTb": _pm(Hf[b].T),
        "WqaT": _pm(Wq[C, :1024].T * s),
        "WqbT": _pm(Wq[C, 1024:].T * s),
        "WkaT": _pm(Wk[C, :1024].T),
        "WkbT": _pm(Wk[C, 1024:].T),
        "WvT": _pm(Wv[C, :].T),
        "WoT": _pm(Wo[:, C].T, bf),
        "GmT": _pm(Gm[b].T, bf),
        "WgT": _pm(Wg.T, bf),
        "bq2": np.ascontiguousarray((bq[C] * s).reshape(2, P).T),
        "bk2": np.ascontiguousarray(bk[C].reshape(2, P).T),
        "bvb": np.ascontiguousarray(np.broadcast_to(bv[C], (P, 256))),
        "bgRow": np.ascontiguousarray(bg.reshape(1, T)).astype(bf),
        "onesRow": np.ones((1, 512), bf),
    }


_NC_CACHE = []


def kernel(**inputs):
    if not _NC_CACHE:
        _NC_CACHE.append(build_nc())
    nc = _NC_CACHE[0]
    in_maps = [_prep_core_inputs(inputs, c) for c in range(8)]
    res = run_bass_kernel_spmd(nc, in_maps, core_ids=list(range(8)))
    bo = np.asarray(inputs["bo"], np.float32)
    out = np.zeros((B, T, D), np.float32)
    for b in range(B):
        acc = np.zeros((T, D), np.float32)
        for g in range(4):
            part = res.results[b * 4 + g]["outp"]  # (128, 16, 1024)
            acc += part.transpose(1, 0, 2).reshape(T, D)
        out[b] = acc + bo[None, :]
    return out



# revision 13
# speedup vs baseline: 1.2282x; 1.2282x over previous
"""Bass/Tile TRN2 kernel for nn_BiasedMultiheadAttention (B=2,T=2048,D=1024,H=16,DM=256).

Sharding: 8 cores = batch(2) x head-group(4).  Each core computes 4 heads of one
batch element plus the (replicated) Gm bias linear, and emits an unnormalized
partial of the output projection; the host sums the 4 partials per batch and
adds bo.

v5 structure (phase B is ACT-paced at ~1.04us per key-block, so everything
that can leave phase B does):
  - all matmul operands bf16 (PSUM accumulation stays f32)
  - X loaded in column chunks [128, 8io, 256] so projection matmuls start
    after the first ~1.5us of DMA instead of after the full half
  - bg folded into the bias-exp via the activation bias operand
  - ALL bias GEMM+exp chains run pre-B where the scalar engine is idle;
    biasE for qc2/qc3 round-trips through DRAM scratch (SBUF holds two
    [128,16,512] biasE buffers)
  - single-bank PSUM tiles on a 2-slot pool ("b2") for V / bias / output
    projection / filler chains -> no long slot-serialization chains
  - ex multiply is one in-place broadcast DVE op per key block
  - output partials in bf16 (host sums in f32)
"""

import numpy as np
import ml_dtypes

import concourse.bass as bass
from concourse.bass import broadcast_tensor_aps
from concourse import bacc
import concourse.mybir as mybir
from concourse.tile import TileContext
from concourse.bass_utils import run_bass_kernel_spmd

B, T, D, H, DM = 2, 2048, 1024, 16, 256
P = 128
F32 = mybir.dt.float32
BF16 = mybir.dt.bfloat16
AF = mybir.ActivationFunctionType
OP = mybir.AluOpType


def _pm(a, dt=np.float32):
    """(R, C) row-major -> partition-major (128, R//128, C), contiguous."""
    a = np.ascontiguousarray(a, dtype=np.float32)
    r, c = a.shape
    return np.ascontiguousarray(a.reshape(r // P, P, c).transpose(1, 0, 2).astype(dt))


def _pm_chunked(a, dt):
    """(R=1024, C=2048) -> (128, 8 col-chunks, 8 io-blocks, 256), so one
    chunk c holds columns [256c, 256c+256) of all 8 row-blocks."""
    x = _pm(a, dt)  # (128, 8, 2048)
    x = x.reshape(P, 8, 8, 256).transpose(0, 2, 1, 3)
    return np.ascontiguousarray(x)


def build_nc():
    nc = bacc.Bacc("TRN2", target_bir_lowering=False, debug=False)

    def inp(name, shape, dt=F32):
        return nc.dram_tensor(name, list(shape), dt, kind="ExternalInput")

    dXa = inp("XTa", (P, 8, 8, 256), BF16)
    dXb = inp("XTb", (P, 8, 8, 256), BF16)
    dWqa = inp("WqaT", (P, 8, 256), BF16)
    dWqb = inp("WqbT", (P, 8, 256), BF16)
    dWka = inp("WkaT", (P, 8, 256), BF16)
    dWkb = inp("WkbT", (P, 8, 256), BF16)
    dWv = inp("WvT", (P, 8, 256), BF16)
    dWo = inp("WoT", (P, 2, 1024), BF16)
    dGm = inp("GmT", (P, 2, 2048), BF16)
    dWg = inp("WgT", (P, 2, 2048), BF16)
    dbq = inp("bq2", (P, 2))
    dbk = inp("bk2", (P, 2))
    dbv = inp("bvb", (P, 256))
    dbg = inp("bgPM", (P, 16))
    dout = nc.dram_tensor("outp", [P, 16, 1024], BF16, kind="ExternalOutput")

    with TileContext(nc) as tc:
        with tc.tile_pool(name="sb", bufs=1) as sb, \
             tc.tile_pool(name="ps", bufs=1, space="PSUM") as ps, \
             tc.tile_pool(name="psb", bufs=2, space="PSUM") as psb, \
             tc.tile_pool(name="pbe", bufs=2) as pbe, \
             tc.tile_pool(name="pstg", bufs=4) as pstg, \
             tc.tile_pool(name="pdr", bufs=1, space="DRAM") as pdr, \
             tc.tile_pool(name="plr", bufs=3) as plr, \
             tc.tile_pool(name="pl", bufs=1) as pl:
            QT = sb.tile([P, 2, 2048], BF16, tag="QT")
            KT = sb.tile([P, 2, 2048], BF16, tag="KT")
            Vt = sb.tile([P, 16, 512], BF16, tag="Vt")  # per tb: [V|1]/[1|V] x2
            oT = sb.tile([P, 2, 2048], BF16, tag="oT")
            woT = sb.tile([P, 2, 1024], BF16, tag="woT")
            gmT = sb.tile([P, 2, 2048], BF16, tag="gmT")
            wgT = sb.tile([P, 2, 2048], BF16, tag="wgT")
            bq_s = sb.tile([P, 2], F32, tag="bq")
            bk_s = sb.tile([P, 2], F32, tag="bk")
            bg_s = sb.tile([P, 16], F32, tag="bg")
            bvb_s = sb.tile([P, 256], F32, tag="bvb")
            wqa_s = sb.tile([P, 8, 256], BF16, tag="wqa")
            wqb_s = sb.tile([P, 8, 256], BF16, tag="wqb")
            wka_s = sb.tile([P, 8, 256], BF16, tag="wka")
            wkb_s = sb.tile([P, 8, 256], BF16, tag="wkb")
            wv_s = sb.tile([P, 8, 256], BF16, tag="wv")
            xa = [sb.tile([P, 8, 256], BF16, tag=f"xa{c}", name=f"xa{c}")
                  for c in range(8)]
            xb = [sb.tile([P, 8, 256], BF16, tag=f"xb{c}", name=f"xb{c}")
                  for c in range(8)]
            dbE = {qc: pdr.tile([P, 16, 512], BF16, tag=f"dbE{qc}",
                                name=f"dbE{qc}")
                   for qc in (2, 3)}

            # --- input DMAs, in priority order ---
            nc.sync.dma_start(bq_s[:], dbq[:])
            nc.sync.dma_start(bk_s[:], dbk[:])
            nc.sync.dma_start(bg_s[:], dbg[:])
            nc.sync.dma_start(bvb_s[:], dbv[:])
            nc.vector.memset(Vt[:], 1.0)
            nc.sync.dma_start(gmT[:], dGm[:])
            nc.sync.dma_start(wgT[:], dWg[:])
            nc.sync.dma_start(wv_s[:], dWv[:])
            for c in range(8):
                nc.sync.dma_start(xa[c][:], dXa[:, c])
            nc.sync.dma_start(wka_s[:], dWka[:])
            nc.sync.dma_start(wqa_s[:], dWqa[:])
            for c in range(8):
                nc.sync.dma_start(xb[c][:], dXb[:, c])
            nc.sync.dma_start(wkb_s[:], dWkb[:])
            nc.sync.dma_start(wqb_s[:], dWqb[:])
            nc.sync.dma_start(woT[:], dWo[:])

            # ---------------- Phase A ----------------
            # V (hx only): V rows for key block tb live in xa chunk tb//2
            for tb in range(16):
                vt = psb.tile([P, 1, 512], F32, tag="b2", name="vt")
                vps = vt[:, 0, 0:256]
                off = (tb % 2) * P
                for io in range(8):
                    nc.tensor.matmul(
                        vps,
                        lhsT=xa[tb // 2][:, io, off:off + P],
                        rhs=wv_s[:, io, :],
                        start=(io == 0), stop=(io == 7),
                    )
                # write data columns of Vt (+bv); ones from memset
                nc.vector.tensor_tensor(
                    Vt[:, tb].rearrange("p (a u v) -> p a u v", a=2, u=4)[:, :, 0:4:3, :],
                    vps.rearrange("p (a u v) -> p a u v", a=2, u=2),
                    bvb_s.rearrange("p (a u v) -> p a u v", a=2, u=2),
                    OP.add,
                )

            def emit_chain(xio, wt, dst, bias_s, qc, tag, first):
                """Full 2-plane projection chain on a 2-bank tag (phase A)."""
                qs = slice(qc * 512, (qc + 1) * 512)
                t = ps.tile([P, 2, 512], F32, tag=tag, name="t")
                for jb in range(2):
                    for cc in range(2):
                        for io in range(8):
                            nc.tensor.matmul(
                                t[:, jb, cc * 256:(cc + 1) * 256],
                                lhsT=wt[:, io, jb * P:(jb + 1) * P],
                                rhs=xio[2 * qc + cc][:, io, :],
                                start=(io == 0), stop=(io == 7),
                            )
                if first:
                    for jb in range(2):
                        nc.vector.tensor_scalar_add(
                            dst[:, jb, qs], t[:, jb, :], bias_s[:, jb:jb + 1])
                else:
                    nc.vector.tensor_tensor(
                        dst[:, :, qs], t[:], dst[:, :, qs], OP.add)

            def emit_chain_b2(xio, wt, dst, qc):
                """Same chain split into per-jb single-bank tiles (B filler)."""
                qs = slice(qc * 512, (qc + 1) * 512)
                for jb in range(2):
                    t1 = psb.tile([P, 1, 512], F32, tag="b2", name="t1")
                    for cc in range(2):
                        for io in range(8):
                            nc.tensor.matmul(
                                t1[:, 0, cc * 256:(cc + 1) * 256],
                                lhsT=wt[:, io, jb * P:(jb + 1) * P],
                                rhs=xio[2 * qc + cc][:, io, :],
                                start=(io == 0), stop=(io == 7),
                            )
                    nc.vector.tensor_tensor(
                        dst[:, jb, qs], t1[:, 0, :], dst[:, jb, qs], OP.add)

            def emit_bias_tb(dst_ap, qct, tb):
                """dst_ap[:, :512] = exp(Wg Gm^T + bg) for key block tb vs
                query chunk qct (single-bank psum, 2-slot pipelining)."""
                tqs = slice(qct * 512, (qct + 1) * 512)
                bps = psb.tile([P, 1, 512], F32, tag="b2", name="bps")
                for ch in range(2):
                    nc.tensor.matmul(
                        bps[:, 0, :],
                        lhsT=wgT[:, ch, tb * P:(tb + 1) * P],
                        rhs=gmT[:, ch, tqs],
                        start=(ch == 0), stop=(ch == 1),
                    )
                nc.scalar.activation(dst_ap, bps[:, 0, :], AF.Exp,
                                     bias=bg_s[:, tb:tb + 1])

            biasEs = {}
            biasEs[0] = pbe.tile([P, 16, 512], BF16, tag="biasE", name="biasE0")
            biasEs[1] = pbe.tile([P, 16, 512], BF16, tag="biasE", name="biasE1")
            for tb in range(16):
                emit_bias_tb(biasEs[0][:, tb, :], 0, tb)
            for qc in range(4):
                emit_chain(xa, wka_s, KT, bk_s, qc, "qk1", True)
            emit_chain(xa, wqa_s, QT, bq_s, 0, "qk0", True)
            for tb in range(16):
                emit_bias_tb(biasEs[1][:, tb, :], 1, tb)
            for qc in range(4):
                emit_chain(xb, wkb_s, KT, bk_s, qc, "qk1", False)
            emit_chain(xb, wqb_s, QT, bq_s, 0, "qk0", False)
            for qct in (2, 3):
                for tb in range(16):
                    stg = pstg.tile([P, 1, 512], BF16, tag="bstg", name="bstg")
                    emit_bias_tb(stg[:, 0, :], qct, tb)
                    nc.sync.dma_start(dbE[qct][:, tb, :], stg[:, 0, :])
                if qct == 2:
                    emit_chain(xa, wqa_s, QT, bq_s, 1, "qk0", True)
                else:
                    emit_chain(xa, wqa_s, QT, bq_s, 2, "qk0", True)
            emit_chain(xa, wqa_s, QT, bq_s, 3, "qk0", True)

            # ---------------- Phase B + C ----------------
            def emit_c(qct, j, jh, tag="b2"):
                qb = 4 * qct + j
                if tag == "b2":
                    po = psb.tile([P, 1, 512], F32, tag=tag, name="po")
                else:
                    po = ps.tile([P, 1, 512], F32, tag=tag, name="po")
                for ch in range(2):
                    nc.tensor.matmul(
                        po[:, 0, :],
                        lhsT=oT[:, ch, qb * P:(qb + 1) * P],
                        rhs=woT[:, ch, jh * 512:(jh + 1) * 512],
                        start=(ch == 0), stop=(ch == 1),
                    )
                osb = plr.tile([P, 1, 512], BF16, tag="osb", name="osb")
                nc.vector.tensor_copy(osb[:], po[:])
                nc.sync.dma_start(dout[:, qb, jh * 512:(jh + 1) * 512],
                                  osb[:, 0, :])

            for qc in range(4):
                qs = slice(qc * 512, (qc + 1) * 512)
                if qc >= 2:
                    # reload this qc's biasE from DRAM into a freed SBUF slot
                    # (chunked so normalization bounce DMAs don't queue
                    # behind one long transfer)
                    bE = pbe.tile([P, 16, 512], BF16, tag="biasE",
                                  name=f"biasE{qc}")
                    for ck in range(4):
                        nc.sync.dma_start(bE[:, 4 * ck:4 * ck + 4, :],
                                          dbE[qc][:, 4 * ck:4 * ck + 4, :])
                    biasEs[qc] = bE
                biasE = biasEs[qc]
                for p in range(2):
                    op = ps.tile([P, 2, 512], F32, tag="out", name="op")
                    for tb in range(16):
                        qk = ps.tile([P, 2, 512], F32, tag=f"qk{tb % 2}",
                                     name=f"qk{tb % 2}")
                        for hh in range(2):
                            dd = slice(hh * 64, (hh + 1) * 64)
                            nc.tensor.matmul(
                                qk[:, hh, :],
                                lhsT=KT[dd, p, tb * P:(tb + 1) * P],
                                rhs=QT[dd, p, qs],
                                start=True, stop=True,
                            )
                        # filler: next qc's second-half Q chain
                        if p == 0 and tb == 2 and qc < 3:
                            emit_chain_b2(xb, wqb_s, QT, qc + 1)
                        ex1 = plr.tile([P, 2, 512], BF16, tag="ex1", name="ex1")
                        nc.scalar.activation(ex1[:], qk[:], AF.Exp)
                        eb_a, eb_b = broadcast_tensor_aps(
                            ex1[:], biasE[:, tb, :].rearrange(
                                "p (a q) -> p a q", a=1))
                        nc.vector.tensor_tensor(ex1[:], eb_a, eb_b, OP.mult)
                        if p == 1 and tb % 4 == 0 and qc > 0:
                            emit_c(qc - 1, tb // 4, 0)
                        if p == 1 and tb % 4 == 2 and qc > 0:
                            emit_c(qc - 1, tb // 4, 1)
                        for hh in range(2):
                            h = 2 * p + hh
                            nc.tensor.matmul(
                                op[:, hh, :],
                                lhsT=Vt[:, tb, h * P:(h + 1) * P],
                                rhs=ex1[:, hh, :],
                                start=(tb == 0), stop=(tb == 15),
                            )
                    # normalize: hh=0 data on parts 0:64 (denom on 64:128);
                    # hh=1 mirrored.  One PSUM->SBUF copy releases the "out"
                    # slot fast; the rest (cross-partition denominator bounce,
                    # reciprocal, scale) runs from SBUF off the critical path.
                    oU = pl.tile([P, 2, 512], F32, tag="oU", name="oU")
                    rB = pl.tile([P, 512], F32, tag="rB", name="rB")
                    rC = pl.tile([P, 512], F32, tag="rC", name="rC")
                    nc.vector.tensor_copy(oU[:], op[:])
                    nc.sync.dma_start(rB[0:64, :], oU[:, 0, :][64:128, :])
                    nc.sync.dma_start(rB[64:128, :], oU[:, 1, :][0:64, :])
                    nc.vector.reciprocal(rC[:], rB[:])
                    nc.vector.tensor_tensor(
                        oT[0:64, p, qs], oU[:, 0, :][0:64, :], rC[0:64, :], OP.mult)
                    nc.vector.tensor_tensor(
                        oT[64:128, p, qs], oU[:, 1, :][64:128, :], rC[64:128, :], OP.mult)

            # phase C for the last qc: every psum tag is free now, so the
            # eight per-jh output chains run widely in parallel
            tags = ["b2", "qk0", "qk1", "out", "b2", "qk0", "qk1", "out"]
            for j in range(4):
                for jh in range(2):
                    emit_c(3, j, jh, tags[2 * j + jh])

    nc.compile()
    return nc


def _prep_core_inputs(inputs, core):
    b, g = core // 4, core % 4
    C = slice(g * 256, (g + 1) * 256)
    Hx = np.asarray(inputs["Hx"], np.float32)
    Hf = np.asarray(inputs["Hf"], np.float32)
    Gm = np.asarray(inputs["Gm"], np.float32)
    Wg = np.asarray(inputs["Wg"], np.float32)
    bg = np.asarray(inputs["bg"], np.float32)
    Wq = np.asarray(inputs["Wq"], np.float32)
    bq = np.asarray(inputs["bq"], np.float32)
    Wk = np.asarray(inputs["Wk"], np.float32)
    bk = np.asarray(inputs["bk"], np.float32)
    Wv = np.asarray(inputs["Wv"], np.float32)
    bv = np.asarray(inputs["bv"], np.float32)
    Wo = np.asarray(inputs["Wo"], np.float32)

    bf = ml_dtypes.bfloat16
    s = 1.0 / 8.0  # 1/sqrt(DK) folded into Q
    return {
        "XTa": _pm_chunked(Hx[b, :, :, 0], bf),
        "XTb": _pm_chunked(Hf[b].T, bf),
        "WqaT": _pm(Wq[C, :1024].T * s, bf),
        "WqbT": _pm(Wq[C, 1024:].T * s, bf),
        "WkaT": _pm(Wk[C, :1024].T, bf),
        "WkbT": _pm(Wk[C, 1024:].T, bf),
        "WvT": _pm(Wv[C, :].T, bf),
        "WoT": _pm(Wo[:, C].T, bf),
        "GmT": _pm(Gm[b].T, bf),
        "WgT": _pm(Wg.T, bf),
        "bq2": np.ascontiguousarray((bq[C] * s).reshape(2, P).T),
        "bk2": np.ascontiguousarray(bk[C].reshape(2, P).T),
        "bvb": np.ascontiguousarray(np.broadcast_to(bv[C], (P, 256))),
        "bgPM": np.ascontiguousarray(bg.reshape(16, P).T),
    }


_NC_CACHE = []


def kernel(**inputs):
    if not _NC_CACHE:
        _NC_CACHE.append(build_nc())
    nc = _NC_CACHE[0]
    in_maps = [_prep_core_inputs(inputs, c) for c in range(8)]
    res = run_bass_kernel_spmd(nc, in_maps, core_ids=list(range(8)))
    bo = np.asarray(inputs["bo"], np.float32)
    out = np.zeros((B, T, D), np.float32)
    for b in range(B):
        acc = np.zeros((T, D), np.float32)
        for g in range(4):
            part = np.asarray(res.results[b * 4 + g]["outp"], np.float32)
            acc += part.transpose(1, 0, 2).reshape(T, D)
        out[b] = acc + bo[None, :]
    return out


# revision 33
# speedup vs baseline: 1.3242x; 1.0782x over previous
"""Bass/Tile TRN2 kernel for nn_BiasedMultiheadAttention (B=2,T=2048,D=1024,H=16,DM=256).

Sharding: 8 cores = batch(2) x head-group(4).  Each core computes 4 heads of one
batch element plus the (replicated) Gm bias linear, and emits an unnormalized
partial of the output projection; the host sums the 4 partials per batch and
adds bo.

v5 structure (phase B is ACT-paced at ~1.04us per key-block, so everything
that can leave phase B does):
  - all matmul operands bf16 (PSUM accumulation stays f32)
  - X loaded in column chunks [128, 8io, 256] so projection matmuls start
    after the first ~1.5us of DMA instead of after the full half
  - bg folded into the bias-exp via the activation bias operand
  - ALL bias GEMM+exp chains run pre-B where the scalar engine is idle;
    biasE for qc2/qc3 round-trips through DRAM scratch (SBUF holds two
    [128,16,512] biasE buffers)
  - single-bank PSUM tiles on a 2-slot pool ("b2") for V / bias / output
    projection / filler chains -> no long slot-serialization chains
  - ex multiply is one in-place broadcast DVE op per key block
  - output partials in bf16 (host sums in f32)
"""

import numpy as np
import ml_dtypes

import concourse.bass as bass
from concourse.bass import broadcast_tensor_aps
from concourse import bacc
import concourse.mybir as mybir
from concourse.tile import TileContext
from concourse.bass_utils import run_bass_kernel_spmd

B, T, D, H, DM = 2, 2048, 1024, 16, 256
P = 128
F32 = mybir.dt.float32
BF16 = mybir.dt.bfloat16
F8 = mybir.dt.float8e4
PM_DR = mybir.MatmulPerfMode.DoubleRow
AF = mybir.ActivationFunctionType
OP = mybir.AluOpType


def _pm(a, dt=np.float32):
    """(R, C) row-major -> partition-major (128, R//128, C), contiguous."""
    a = np.ascontiguousarray(a, dtype=np.float32)
    r, c = a.shape
    return np.ascontiguousarray(a.reshape(r // P, P, c).transpose(1, 0, 2).astype(dt))


def _pm_chunked(a, dt):
    """(R=1024, C=2048) -> (128, 8 col-chunks, 8 io-blocks, 256), so one
    chunk c holds columns [256c, 256c+256) of all 8 row-blocks."""
    x = _pm(a, dt)  # (128, 8, 2048)
    x = x.reshape(P, 8, 8, 256).transpose(0, 2, 1, 3)
    return np.ascontiguousarray(x)


def build_nc():
    nc = bacc.Bacc("TRN2", target_bir_lowering=False, debug=False)

    def inp(name, shape, dt=F32):
        return nc.dram_tensor(name, list(shape), dt, kind="ExternalInput")

    dXa = inp("XTa", (P, 8, 8, 256), BF16)
    dXb = inp("XTb", (P, 8, 8, 256), BF16)
    dWqa = inp("WqaT", (P, 8, 256), BF16)
    dWqb = inp("WqbT", (P, 8, 256), BF16)
    dWka = inp("WkaT", (P, 8, 256), BF16)
    dWkb = inp("WkbT", (P, 8, 256), BF16)
    dWv = inp("WvT", (P, 8, 256), BF16)
    dWo = inp("WoT", (P, 2, 1024), BF16)
    dGm = inp("GmT", (P, 2, 2048), F8)
    dWg = inp("WgT", (P, 2, 2048), F8)
    dsm = inp("smalls", (P, 276))  # [bq2 | bk2 | bgPM | bvb]
    dout = nc.dram_tensor("outp", [P, 16, 1024], BF16, kind="ExternalOutput")

    with TileContext(nc) as tc:
        with tc.tile_pool(name="sb", bufs=1) as sb, \
             tc.tile_pool(name="ps", bufs=1, space="PSUM") as ps, \
             tc.tile_pool(name="psb", bufs=2, space="PSUM") as psb, \
             tc.tile_pool(name="pbe", bufs=2) as pbe, \
             tc.tile_pool(name="pstg", bufs=4) as pstg, \
             tc.tile_pool(name="pdr", bufs=1, space="DRAM") as pdr, \
             tc.tile_pool(name="plr", bufs=3) as plr, \
             tc.tile_pool(name="pl", bufs=1) as pl:
            QT = sb.tile([P, 2, 2048], BF16, tag="QT")
            KT = sb.tile([P, 2, 2048], BF16, tag="KT")
            Vt = sb.tile([P, 16, 512], BF16, tag="Vt")  # per tb: [V|1]/[1|V] x2
            oT = sb.tile([P, 2, 2048], BF16, tag="oT")
            woT = sb.tile([P, 2, 1024], BF16, tag="woT")
            gmT = sb.tile([P, 2, 2048], F8, tag="gmT")
            wgT = sb.tile([P, 2, 2048], F8, tag="wgT")
            sm = sb.tile([P, 276], F32, tag="sm")
            bq_s = sm[:, 0:2]
            bk_s = sm[:, 2:4]
            bg_s = sm[:, 4:20]
            bvb_s = sm[:, 20:276]
            wqa_s = sb.tile([P, 8, 256], BF16, tag="wqa")
            wqb_s = sb.tile([P, 8, 256], BF16, tag="wqb")
            wka_s = sb.tile([P, 8, 256], BF16, tag="wka")
            wkb_s = sb.tile([P, 8, 256], BF16, tag="wkb")
            wv_s = sb.tile([P, 8, 256], BF16, tag="wv")
            xa = [sb.tile([P, 8, 256], BF16, tag=f"xa{c}", name=f"xa{c}")
                  for c in range(8)]
            xb = [sb.tile([P, 8, 256], BF16, tag=f"xb{c}", name=f"xb{c}")
                  for c in range(8)]
            dbE = {qc: pdr.tile([P, 16, 512], BF16, tag=f"dbE{qc}",
                                name=f"dbE{qc}")
                   for qc in (2, 3)}

            # --- input DMAs, in priority order; gm/wg chunked so the first
            # bias chains start ~5us in ---
            nc.sync.dma_start(sm[:], dsm[:])
            nc.vector.memset(
                Vt.rearrange("p t (a v) -> p (t a) v", a=2)[:, :, 64:192], 1.0)
            nc.sync.dma_start(gmT[:, :, 0:512], dGm[:, :, 0:512])
            nc.sync.dma_start(wgT[:, :, 0:1024], dWg[:, :, 0:1024])
            nc.sync.dma_start(wgT[:, :, 1024:2048], dWg[:, :, 1024:2048])
            nc.sync.dma_start(gmT[:, :, 512:2048], dGm[:, :, 512:2048])
            nc.sync.dma_start(wv_s[:], dWv[:])
            for c in range(8):
                nc.sync.dma_start(xa[c][:], dXa[:, c])
            nc.sync.dma_start(wka_s[:], dWka[:])
            nc.sync.dma_start(wqa_s[:], dWqa[:])
            for c in range(8):
                nc.sync.dma_start(xb[c][:], dXb[:, c])
            nc.sync.dma_start(wkb_s[:], dWkb[:])
            nc.sync.dma_start(wqb_s[:], dWqb[:])
            nc.sync.dma_start(woT[:], dWo[:])

            # ---------------- Phase A ----------------
            def emit_bias_tb(dst_ap, qct, tb):
                """dst_ap[:, :512] = exp(Wg Gm^T + bg) for key block tb vs
                query chunk qct.  fp8 DoubleRow folds both DM k-subtiles
                into one half-rate matmul (single-bank psum, 2-slot
                pipelining)."""
                tqs = slice(qct * 512, (qct + 1) * 512)
                bps = psb.tile([P, 1, 512], F32, tag="b2", name="bps")
                nc.tensor.matmul(
                    bps[:, 0, :],
                    lhsT=wgT[:, :, tb * P:(tb + 1) * P],
                    rhs=gmT[:, :, tqs],
                    start=True, stop=True,
                    perf_mode=PM_DR,
                )
                nc.scalar.activation(dst_ap, bps[:, 0, :], AF.Exp,
                                     bias=bg_s[:, tb:tb + 1])

            biasEs = {}
            biasEs[0] = pbe.tile([P, 16, 512], BF16, tag="biasE", name="biasE0")
            biasEs[1] = pbe.tile([P, 16, 512], BF16, tag="biasE", name="biasE1")
            for tb in range(16):
                emit_bias_tb(biasEs[0][:, tb, :], 0, tb)

            # V (hx only): V rows for key block tb live in xa chunk tb//2
            for tb in range(16):
                vt = psb.tile([P, 1, 512], F32, tag="b2", name="vt")
                vps = vt[:, 0, 0:256]
                off = (tb % 2) * P
                for io in range(8):
                    nc.tensor.matmul(
                        vps,
                        lhsT=xa[tb // 2][:, io, off:off + P],
                        rhs=wv_s[:, io, :],
                        start=(io == 0), stop=(io == 7),
                    )
                # write data columns of Vt (+bv); ones from memset
                nc.vector.tensor_tensor(
                    Vt[:, tb].rearrange("p (a u v) -> p a u v", a=2, u=4)[:, :, 0:4:3, :],
                    vps.rearrange("p (a u v) -> p a u v", a=2, u=2),
                    bvb_s.rearrange("p (a u v) -> p a u v", a=2, u=2),
                    OP.add,
                )

            def emit_chain(xio, wt, dst, bias_s, qc, tag, first):
                """Full 2-plane projection chain on a 2-bank tag (phase A)."""
                qs = slice(qc * 512, (qc + 1) * 512)
                t = ps.tile([P, 2, 512], F32, tag=tag, name="t")
                for jb in range(2):
                    for cc in range(2):
                        for io in range(8):
                            nc.tensor.matmul(
                                t[:, jb, cc * 256:(cc + 1) * 256],
                                lhsT=wt[:, io, jb * P:(jb + 1) * P],
                                rhs=xio[2 * qc + cc][:, io, :],
                                start=(io == 0), stop=(io == 7),
                            )
                if first:
                    for jb in range(2):
                        nc.vector.tensor_scalar_add(
                            dst[:, jb, qs], t[:, jb, :], bias_s[:, jb:jb + 1])
                else:
                    nc.vector.tensor_tensor(
                        dst[:, :, qs], t[:], dst[:, :, qs], OP.add)

            def emit_chain_b2(xio, wt, dst, qc):
                """Same chain split into per-jb single-bank tiles (B filler)."""
                qs = slice(qc * 512, (qc + 1) * 512)
                for jb in range(2):
                    t1 = psb.tile([P, 1, 512], F32, tag="b2", name="t1")
                    for cc in range(2):
                        for io in range(8):
                            nc.tensor.matmul(
                                t1[:, 0, cc * 256:(cc + 1) * 256],
                                lhsT=wt[:, io, jb * P:(jb + 1) * P],
                                rhs=xio[2 * qc + cc][:, io, :],
                                start=(io == 0), stop=(io == 7),
                            )
                    nc.vector.tensor_tensor(
                        dst[:, jb, qs], t1[:, 0, :], dst[:, jb, qs], OP.add)

            for qc in range(4):
                emit_chain(xa, wka_s, KT, bk_s, qc, "qk1", True)
            emit_chain(xa, wqa_s, QT, bq_s, 0, "qk0", True)
            for tb in range(16):
                emit_bias_tb(biasEs[1][:, tb, :], 1, tb)
            for qc in range(4):
                emit_chain(xb, wkb_s, KT, bk_s, qc, "qk1", False)
            emit_chain(xb, wqb_s, QT, bq_s, 0, "qk0", False)
            for qct in (2, 3):
                for tg in range(4):
                    # 4 exps share one staging tile -> one DMA to DRAM
                    # scratch per group (per-tb DMAs backpressure the exps)
                    stg = pstg.tile([P, 4, 512], BF16, tag="bstg", name="bstg")
                    for ti in range(4):
                        emit_bias_tb(stg[:, ti, :], qct, 4 * tg + ti)
                    nc.sync.dma_start(dbE[qct][:, 4 * tg:4 * tg + 4, :], stg[:])
                if qct == 2:
                    emit_chain(xa, wqa_s, QT, bq_s, 1, "qk0", True)
                else:
                    emit_chain(xa, wqa_s, QT, bq_s, 2, "qk0", True)
            emit_chain(xa, wqa_s, QT, bq_s, 3, "qk0", True)

            # ---------------- Phase B + C ----------------
            def emit_c(qct, j, jh, tag="b2"):
                qb = 4 * qct + j
                if tag == "b2":
                    po = psb.tile([P, 1, 512], F32, tag=tag, name="po")
                else:
                    po = ps.tile([P, 1, 512], F32, tag=tag, name="po")
                for ch in range(2):
                    nc.tensor.matmul(
                        po[:, 0, :],
                        lhsT=oT[:, ch, qb * P:(qb + 1) * P],
                        rhs=woT[:, ch, jh * 512:(jh + 1) * 512],
                        start=(ch == 0), stop=(ch == 1),
                    )
                osb = plr.tile([P, 1, 512], BF16, tag="osb", name="osb")
                nc.vector.tensor_copy(osb[:], po[:])
                nc.sync.dma_start(dout[:, qb, jh * 512:(jh + 1) * 512],
                                  osb[:, 0, :])

            for qc in range(4):
                qs = slice(qc * 512, (qc + 1) * 512)
                if qc >= 2:
                    # reload this qc's biasE from DRAM into a freed SBUF slot
                    # (chunked so normalization bounce DMAs don't queue
                    # behind one long transfer)
                    bE = pbe.tile([P, 16, 512], BF16, tag="biasE",
                                  name=f"biasE{qc}")
                    for ck in range(4):
                        nc.sync.dma_start(bE[:, 4 * ck:4 * ck + 4, :],
                                          dbE[qc][:, 4 * ck:4 * ck + 4, :])
                    biasEs[qc] = bE
                biasE = biasEs[qc]
                for p in range(2):
                    op = ps.tile([P, 2, 512], F32, tag="out", name="op")
                    for tb in range(16):
                        qk = ps.tile([P, 2, 512], F32, tag=f"qk{tb % 2}",
                                     name=f"qk{tb % 2}")
                        for hh in range(2):
                            dd = slice(hh * 64, (hh + 1) * 64)
                            nc.tensor.matmul(
                                qk[:, hh, :],
                                lhsT=KT[dd, p, tb * P:(tb + 1) * P],
                                rhs=QT[dd, p, qs],
                                start=True, stop=True,
                            )
                        # filler: next qc's second-half Q chain
                        if p == 0 and tb == 2 and qc < 3:
                            emit_chain_b2(xb, wqb_s, QT, qc + 1)
                        ex1 = plr.tile([P, 2, 512], BF16, tag="ex1", name="ex1")
                        nc.scalar.activation(ex1[:], qk[:], AF.Exp)
                        eb_a, eb_b = broadcast_tensor_aps(
                            ex1[:], biasE[:, tb, :].rearrange(
                                "p (a q) -> p a q", a=1))
                        nc.vector.tensor_tensor(ex1[:], eb_a, eb_b, OP.mult)
                        if p == 1 and tb % 4 == 0 and qc > 0:
                            emit_c(qc - 1, tb // 4, 0)
                        if p == 1 and tb % 4 == 2 and qc > 0:
                            emit_c(qc - 1, tb // 4, 1)
                        for hh in range(2):
                            h = 2 * p + hh
                            nc.tensor.matmul(
                                op[:, hh, :],
                                lhsT=Vt[:, tb, h * P:(h + 1) * P],
                                rhs=ex1[:, hh, :],
                                start=(tb == 0), stop=(tb == 15),
                            )
                    # normalize: hh=0 data on parts 0:64 (denom on 64:128);
                    # hh=1 mirrored.  Engines can't cross partitions, so the
                    # denominators bounce through SBUF via DMA.
                    rB = pl.tile([P, 512], F32, tag="rB", name="rB")
                    rC = pl.tile([P, 512], F32, tag="rC", name="rC")
                    if qc == 3 and p == 1:
                        # last iteration: nothing else needs the "out" slot,
                        # so skip the big PSUM->SBUF copy and run the
                        # shortest-latency chain straight off the accumulator
                        t0 = pl.tile([P, 512], F32, tag="t0", name="t0")
                        nc.vector.tensor_copy(t0[64:128, :], op[:, 0, :][64:128, :])
                        nc.scalar.activation(t0[0:64, :], op[:, 1, :][0:64, :],
                                             AF.Copy)
                        nc.sync.dma_start(rB[0:64, :], t0[64:128, :])
                        nc.sync.dma_start(rB[64:128, :], t0[0:64, :])
                        nc.vector.reciprocal(rC[:], rB[:])
                        nc.vector.tensor_tensor(
                            oT[0:64, p, qs], op[:, 0, :][0:64, :],
                            rC[0:64, :], OP.mult)
                        nc.vector.tensor_tensor(
                            oT[64:128, p, qs], op[:, 1, :][64:128, :],
                            rC[64:128, :], OP.mult)
                    else:
                        # steady state: one PSUM->SBUF copy releases the
                        # "out" slot fast; the rest of the chain runs from
                        # SBUF off the accumulator's critical path, with the
                        # scale ops on the otherwise-idle gpsimd engine
                        oU = pl.tile([P, 2, 512], F32, tag="oU", name="oU")
                        nc.vector.tensor_copy(oU[:], op[:])
                        nc.sync.dma_start(rB[0:64, :], oU[:, 0, :][64:128, :])
                        nc.sync.dma_start(rB[64:128, :], oU[:, 1, :][0:64, :])
                        nc.vector.reciprocal(rC[:], rB[:])
                        nc.gpsimd.tensor_tensor(
                            oT[0:64, p, qs], oU[:, 0, :][0:64, :],
                            rC[0:64, :], OP.mult)
                        nc.gpsimd.tensor_tensor(
                            oT[64:128, p, qs], oU[:, 1, :][64:128, :],
                            rC[64:128, :], OP.mult)

            # phase C for the last qc: the first-half (ch=0) matmuls depend
            # only on the p=0 rows of oT, so they run DURING the final
            # normalization chain and keep PE warm; the ch=1 halves follow
            # right after it.  Copies alternate between DVE and the now-idle
            # scalar engine; DMAs pipeline behind them.
            tags = ["b2", "qk0", "qk1", "out", "b2", "qk0", "qk1", "out"]
            pos = []
            for j in range(4):
                for jh in range(2):
                    qb = 12 + j
                    po = (psb if tags[2 * j + jh] == "b2" else ps).tile(
                        [P, 1, 512], F32, tag=tags[2 * j + jh], name="po")
                    nc.tensor.matmul(
                        po[:, 0, :],
                        lhsT=oT[:, 0, qb * P:(qb + 1) * P],
                        rhs=woT[:, 0, jh * 512:(jh + 1) * 512],
                        start=True, stop=False,
                    )
                    pos.append((po, j, jh, qb))
            for idx, (po, j, jh, qb) in enumerate(pos):
                nc.tensor.matmul(
                    po[:, 0, :],
                    lhsT=oT[:, 1, qb * P:(qb + 1) * P],
                    rhs=woT[:, 1, jh * 512:(jh + 1) * 512],
                    start=False, stop=True,
                )
                osb = plr.tile([P, 1, 512], BF16, tag="osb", name="osb")
                if idx % 2:
                    nc.scalar.activation(osb[:], po[:], AF.Copy)
                else:
                    nc.vector.tensor_copy(osb[:], po[:])
                nc.sync.dma_start(dout[:, qb, jh * 512:(jh + 1) * 512],
                                  osb[:, 0, :])

    nc.compile()
    return nc


def _prep_core_inputs(inputs, core):
    b, g = core // 4, core % 4
    C = slice(g * 256, (g + 1) * 256)
    Hx = np.asarray(inputs["Hx"], np.float32)
    Hf = np.asarray(inputs["Hf"], np.float32)
    Gm = np.asarray(inputs["Gm"], np.float32)
    Wg = np.asarray(inputs["Wg"], np.float32)
    bg = np.asarray(inputs["bg"], np.float32)
    Wq = np.asarray(inputs["Wq"], np.float32)
    bq = np.asarray(inputs["bq"], np.float32)
    Wk = np.asarray(inputs["Wk"], np.float32)
    bk = np.asarray(inputs["bk"], np.float32)
    Wv = np.asarray(inputs["Wv"], np.float32)
    bv = np.asarray(inputs["bv"], np.float32)
    Wo = np.asarray(inputs["Wo"], np.float32)

    bf = ml_dtypes.bfloat16
    s = 1.0 / 8.0  # 1/sqrt(DK) folded into Q
    return {
        "XTa": _pm_chunked(Hx[b, :, :, 0], bf),
        "XTb": _pm_chunked(Hf[b].T, bf),
        "WqaT": _pm(Wq[C, :1024].T * s, bf),
        "WqbT": _pm(Wq[C, 1024:].T * s, bf),
        "WkaT": _pm(Wk[C, :1024].T, bf),
        "WkbT": _pm(Wk[C, 1024:].T, bf),
        "WvT": _pm(Wv[C, :].T, bf),
        "WoT": _pm(Wo[:, C].T, bf),
        "GmT": _pm(Gm[b].T, ml_dtypes.float8_e4m3),
        "WgT": _pm(Wg.T, ml_dtypes.float8_e4m3),
        "smalls": np.ascontiguousarray(np.concatenate([
            (bq[C] * s).reshape(2, P).T,
            bk[C].reshape(2, P).T,
            bg.reshape(16, P).T,
            np.broadcast_to(bv[C], (P, 256)),
        ], axis=1, dtype=np.float32)),
    }


_NC_CACHE = []


def kernel(**inputs):
    if not _NC_CACHE:
        _NC_CACHE.append(build_nc())
    nc = _NC_CACHE[0]
    in_maps = [_prep_core_inputs(inputs, c) for c in range(8)]
    res = run_bass_kernel_spmd(nc, in_maps, core_ids=list(range(8)))
    bo = np.asarray(inputs["bo"], np.float32)
    out = np.zeros((B, T, D), np.float32)
    for b in range(B):
        acc = np.zeros((T, D), np.float32)
        for g in range(4):
            part = np.asarray(res.results[b * 4 + g]["outp"], np.float32)
            acc += part.transpose(1, 0, 2).reshape(T, D)
        out[b] = acc + bo[None, :]
    return out


# revision 42
# speedup vs baseline: 1.3647x; 1.0306x over previous
"""Bass/Tile TRN2 kernel for nn_BiasedMultiheadAttention (B=2,T=2048,D=1024,H=16,DM=256).

Sharding: 8 cores = batch(2) x head-group(4).  Each core computes 4 heads of one
batch element plus the (replicated) Gm bias linear, and emits an unnormalized
partial of the output projection; the host sums the 4 partials per batch and
adds bo.

v5 structure (phase B is ACT-paced at ~1.04us per key-block, so everything
that can leave phase B does):
  - all matmul operands bf16 (PSUM accumulation stays f32)
  - X loaded in column chunks [128, 8io, 256] so projection matmuls start
    after the first ~1.5us of DMA instead of after the full half
  - bg folded into the bias-exp via the activation bias operand
  - ALL bias GEMM+exp chains run pre-B where the scalar engine is idle;
    biasE for qc2/qc3 round-trips through DRAM scratch (SBUF holds two
    [128,16,512] biasE buffers)
  - single-bank PSUM tiles on a 2-slot pool ("b2") for V / bias / output
    projection / filler chains -> no long slot-serialization chains
  - ex multiply is one in-place broadcast DVE op per key block
  - output partials in bf16 (host sums in f32)
"""

import numpy as np
import ml_dtypes

import concourse.bass as bass
from concourse.bass import broadcast_tensor_aps
from concourse import bacc
import concourse.mybir as mybir
from concourse.tile import TileContext
from concourse.bass_utils import run_bass_kernel_spmd

B, T, D, H, DM = 2, 2048, 1024, 16, 256
P = 128
F32 = mybir.dt.float32
BF16 = mybir.dt.bfloat16
F8 = mybir.dt.float8e4
PM_DR = mybir.MatmulPerfMode.DoubleRow
AF = mybir.ActivationFunctionType
OP = mybir.AluOpType


def _pm(a, dt=np.float32):
    """(R, C) row-major -> partition-major (128, R//128, C), contiguous."""
    a = np.ascontiguousarray(a, dtype=np.float32)
    r, c = a.shape
    return np.ascontiguousarray(a.reshape(r // P, P, c).transpose(1, 0, 2).astype(dt))


def _pm_chunked(a, dt):
    """(R=1024, C=2048) -> (128, 4 col-chunks, 8 io-blocks, 512), so one
    chunk c holds columns [512c, 512c+512) of all 8 row-blocks."""
    x = _pm(a, dt)  # (128, 8, 2048)
    x = x.reshape(P, 8, 4, 512).transpose(0, 2, 1, 3)
    return np.ascontiguousarray(x)


def build_nc():
    nc = bacc.Bacc("TRN2", target_bir_lowering=False, debug=False)

    def inp(name, shape, dt=F32):
        return nc.dram_tensor(name, list(shape), dt, kind="ExternalInput")

    dXa = inp("XTa", (P, 4, 8, 512), BF16)
    dXb = inp("XTb", (P, 4, 8, 512), BF16)
    dWqa = inp("WqaT", (P, 8, 256), BF16)
    dWqb = inp("WqbT", (P, 8, 256), BF16)
    dWka = inp("WkaT", (P, 8, 256), BF16)
    dWkb = inp("WkbT", (P, 8, 256), BF16)
    dWv = inp("WvT", (P, 8, 256), BF16)
    dWo = inp("WoT", (P, 2, 1024), BF16)
    dGm = inp("GmT", (P, 2, 2048), F8)
    dWg = inp("WgT", (P, 2, 2048), F8)
    dsm = inp("smalls", (P, 276))  # [bq2 | bk2 | bgPM | bvb]
    dout = nc.dram_tensor("outp", [P, 16, 1024], BF16, kind="ExternalOutput")

    with TileContext(nc) as tc:
        with tc.tile_pool(name="sb", bufs=1) as sb, \
             tc.tile_pool(name="ps", bufs=1, space="PSUM") as ps, \
             tc.tile_pool(name="psb", bufs=2, space="PSUM") as psb, \
             tc.tile_pool(name="pbe", bufs=2) as pbe, \
             tc.tile_pool(name="pstg", bufs=3) as pstg, \
             tc.tile_pool(name="pdr", bufs=1, space="DRAM") as pdr, \
             tc.tile_pool(name="plr", bufs=3) as plr, \
             tc.tile_pool(name="pl", bufs=1) as pl:
            QT = sb.tile([P, 2, 2048], BF16, tag="QT")
            KT = sb.tile([P, 2, 2048], BF16, tag="KT")
            Vt = sb.tile([P, 16, 512], BF16, tag="Vt")  # per tb: [V|1]/[1|V] x2
            oT = sb.tile([P, 2, 2048], BF16, tag="oT")
            woT = sb.tile([P, 2, 1024], BF16, tag="woT")
            gmT = sb.tile([P, 2, 2048], F8, tag="gmT")
            wgT = sb.tile([P, 2, 2048], F8, tag="wgT")
            sm = sb.tile([P, 276], F32, tag="sm")
            bq_s = sm[:, 0:2]
            bk_s = sm[:, 2:4]
            bg_s = sm[:, 4:20]
            bvb_s = sm[:, 20:276]
            wqa_s = sb.tile([P, 8, 256], BF16, tag="wqa")
            wqb_s = sb.tile([P, 8, 256], BF16, tag="wqb")
            wka_s = sb.tile([P, 8, 256], BF16, tag="wka")
            wkb_s = sb.tile([P, 8, 256], BF16, tag="wkb")
            wv_s = sb.tile([P, 8, 256], BF16, tag="wv")
            xa = [sb.tile([P, 8, 512], BF16, tag=f"xa{c}", name=f"xa{c}")
                  for c in range(4)]
            xb = [sb.tile([P, 8, 512], BF16, tag=f"xb{c}", name=f"xb{c}")
                  for c in range(4)]
            dbE = {qc: pdr.tile([P, 16, 512], BF16, tag=f"dbE{qc}",
                                name=f"dbE{qc}")
                   for qc in (2, 3)}

            # --- input DMAs, in priority order; gm/wg chunked so the first
            # bias chains start ~5us in ---
            nc.sync.dma_start(sm[:], dsm[:])
            nc.vector.memset(
                Vt.rearrange("p t (a v) -> p (t a) v", a=2)[:, :, 64:192], 1.0)
            nc.sync.dma_start(gmT[:, :, 0:512], dGm[:, :, 0:512])
            nc.sync.dma_start(wgT[:, :, 0:1024], dWg[:, :, 0:1024])
            nc.sync.dma_start(wgT[:, :, 1024:2048], dWg[:, :, 1024:2048])
            nc.sync.dma_start(gmT[:, :, 512:2048], dGm[:, :, 512:2048])
            nc.sync.dma_start(xa[0][:], dXa[:, 0])
            nc.sync.dma_start(wka_s[:], dWka[:])
            nc.sync.dma_start(wqa_s[:], dWqa[:])
            nc.sync.dma_start(wv_s[:], dWv[:])
            for c in range(1, 4):
                nc.sync.dma_start(xa[c][:], dXa[:, c])
            for c in range(4):
                nc.sync.dma_start(xb[c][:], dXb[:, c])
            nc.sync.dma_start(wkb_s[:], dWkb[:])
            nc.sync.dma_start(wqb_s[:], dWqb[:])
            nc.sync.dma_start(woT[:], dWo[:])

            # ---------------- Phase A ----------------
            def emit_bias_tb(dst_ap, qct, tb):
                """dst_ap[:, :512] = exp(Wg Gm^T + bg) for key block tb vs
                query chunk qct.  fp8 DoubleRow folds both DM k-subtiles
                into one half-rate matmul (single-bank psum, 2-slot
                pipelining)."""
                tqs = slice(qct * 512, (qct + 1) * 512)
                bps = psb.tile([P, 1, 512], F32, tag="b2", name="bps")
                nc.tensor.matmul(
                    bps[:, 0, :],
                    lhsT=wgT[:, :, tb * P:(tb + 1) * P],
                    rhs=gmT[:, :, tqs],
                    start=True, stop=True,
                    perf_mode=PM_DR,
                )
                nc.scalar.activation(dst_ap, bps[:, 0, :], AF.Exp,
                                     bias=bg_s[:, tb:tb + 1])

            def emit_v(tb):
                """V chain for key block tb (hx only; xa chunk tb//4)."""
                vt = psb.tile([P, 1, 512], F32, tag="b2", name="vt")
                vps = vt[:, 0, 0:256]
                off = (tb % 4) * P
                for io in range(8):
                    nc.tensor.matmul(
                        vps,
                        lhsT=xa[tb // 4][:, io, off:off + P],
                        rhs=wv_s[:, io, :],
                        start=(io == 0), stop=(io == 7),
                    )
                # write data columns of Vt (+bv); ones from memset
                nc.vector.tensor_tensor(
                    Vt[:, tb].rearrange("p (a u v) -> p a u v", a=2, u=4)[:, :, 0:4:3, :],
                    vps.rearrange("p (a u v) -> p a u v", a=2, u=2),
                    bvb_s.rearrange("p (a u v) -> p a u v", a=2, u=2),
                    OP.add,
                )

            def emit_chain(xio, wt, dst, bias_s, qc, tag, first):
                """Full 2-plane projection chain on a 2-bank tag (phase A)."""
                qs = slice(qc * 512, (qc + 1) * 512)
                t = ps.tile([P, 2, 512], F32, tag=tag, name="t")
                for jb in range(2):
                    for io in range(8):
                        nc.tensor.matmul(
                            t[:, jb, :],
                            lhsT=wt[:, io, jb * P:(jb + 1) * P],
                            rhs=xio[qc][:, io, :],
                            start=(io == 0), stop=(io == 7),
                        )
                if first:
                    for jb in range(2):
                        nc.vector.tensor_scalar_add(
                            dst[:, jb, qs], t[:, jb, :], bias_s[:, jb:jb + 1])
                else:
                    nc.vector.tensor_tensor(
                        dst[:, :, qs], t[:], dst[:, :, qs], OP.add)

            def emit_chain_b2(xio, wt, dst, bias_s, qc, first):
                """Same chain split into per-jb single-bank tiles (B filler)."""
                qs = slice(qc * 512, (qc + 1) * 512)
                for jb in range(2):
                    t1 = psb.tile([P, 1, 512], F32, tag="b2", name="t1")
                    for io in range(8):
                        nc.tensor.matmul(
                            t1[:, 0, :],
                            lhsT=wt[:, io, jb * P:(jb + 1) * P],
                            rhs=xio[qc][:, io, :],
                            start=(io == 0), stop=(io == 7),
                        )
                    if first:
                        nc.vector.tensor_scalar_add(
                            dst[:, jb, qs], t1[:, 0, :], bias_s[:, jb:jb + 1])
                    else:
                        nc.vector.tensor_tensor(
                            dst[:, jb, qs], t1[:, 0, :], dst[:, jb, qs], OP.add)

            biasEs = {}
            biasEs[0] = pbe.tile([P, 16, 512], BF16, tag="biasE", name="biasE0")
            biasEs[1] = pbe.tile([P, 16, 512], BF16, tag="biasE", name="biasE1")
            for tb in range(16):
                emit_bias_tb(biasEs[0][:, tb, :], 0, tb)
            for tb in range(16):
                emit_bias_tb(biasEs[1][:, tb, :], 1, tb)
            for qct in (2, 3):
                for tg in range(4):
                    # 4 exps share one staging tile -> one DMA to DRAM
                    # scratch per group
                    stg = pstg.tile([P, 4, 512], BF16, tag="bstg", name="bstg")
                    for ti in range(4):
                        emit_bias_tb(stg[:, ti, :], qct, 4 * tg + ti)
                    nc.sync.dma_start(dbE[qct][:, 4 * tg:4 * tg + 4, :], stg[:])
            for qc in range(4):
                emit_chain(xa, wka_s, KT, bk_s, qc, "qk1", True)
            emit_chain(xa, wqa_s, QT, bq_s, 0, "qk0", True)
            for qc in range(4):
                emit_chain(xb, wkb_s, KT, bk_s, qc, "qk1", False)
            emit_chain(xb, wqb_s, QT, bq_s, 0, "qk0", False)
            for tb in range(6):
                emit_v(tb)

            # ---------------- Phase B + C ----------------
            def emit_c(qct, j, jh, tag="b2"):
                qb = 4 * qct + j
                if tag == "b2":
                    po = psb.tile([P, 1, 512], F32, tag=tag, name="po")
                else:
                    po = ps.tile([P, 1, 512], F32, tag=tag, name="po")
                for ch in range(2):
                    nc.tensor.matmul(
                        po[:, 0, :],
                        lhsT=oT[:, ch, qb * P:(qb + 1) * P],
                        rhs=woT[:, ch, jh * 512:(jh + 1) * 512],
                        start=(ch == 0), stop=(ch == 1),
                    )
                osb = plr.tile([P, 1, 512], BF16, tag="osb", name="osb")
                nc.vector.tensor_copy(osb[:], po[:])
                nc.sync.dma_start(dout[:, qb, jh * 512:(jh + 1) * 512],
                                  osb[:, 0, :])

            for qc in range(4):
                qs = slice(qc * 512, (qc + 1) * 512)
                if qc >= 2:
                    # reload this qc's biasE from DRAM into a freed SBUF slot
                    # (chunked so normalization bounce DMAs don't queue
                    # behind one long transfer)
                    bE = pbe.tile([P, 16, 512], BF16, tag="biasE",
                                  name=f"biasE{qc}")
                    for ck in range(4):
                        nc.sync.dma_start(bE[:, 4 * ck:4 * ck + 4, :],
                                          dbE[qc][:, 4 * ck:4 * ck + 4, :])
                    biasEs[qc] = bE
                biasE = biasEs[qc]
                for p in range(2):
                    op = ps.tile([P, 2, 512], F32, tag="out", name="op")
                    for tb in range(16):
                        qk = ps.tile([P, 2, 512], F32, tag=f"qk{tb % 2}",
                                     name=f"qk{tb % 2}")
                        for hh in range(2):
                            dd = slice(hh * 64, (hh + 1) * 64)
                            nc.tensor.matmul(
                                qk[:, hh, :],
                                lhsT=KT[dd, p, tb * P:(tb + 1) * P],
                                rhs=QT[dd, p, qs],
                                start=True, stop=True,
                            )
                        # fillers that keep PE fed under the ACT exp pace:
                        # remaining V chains (qc0), the next qc's Q chains,
                        # and the previous qc's output projections
                        if qc == 0 and p == 0 and tb < 10:
                            emit_v(tb + 6)
                        if p == 0 and tb == 2 and qc < 3:
                            emit_chain_b2(xa, wqa_s, QT, bq_s, qc + 1, True)
                        if p == 0 and tb == 12 and qc < 3:
                            emit_chain_b2(xb, wqb_s, QT, bq_s, qc + 1, False)
                        ex1 = plr.tile([P, 2, 512], BF16, tag="ex1", name="ex1")
                        nc.scalar.activation(ex1[:], qk[:], AF.Exp)
                        eb_a, eb_b = broadcast_tensor_aps(
                            ex1[:], biasE[:, tb, :].rearrange(
                                "p (a q) -> p a q", a=1))
                        nc.vector.tensor_tensor(ex1[:], eb_a, eb_b, OP.mult)
                        if p == 1 and tb % 4 == 0 and qc > 0:
                            emit_c(qc - 1, tb // 4, 0)
                        if p == 1 and tb % 4 == 2 and qc > 0:
                            emit_c(qc - 1, tb // 4, 1)
                        for hh in range(2):
                            h = 2 * p + hh
                            nc.tensor.matmul(
                                op[:, hh, :],
                                lhsT=Vt[:, tb, h * P:(h + 1) * P],
                                rhs=ex1[:, hh, :],
                                start=(tb == 0), stop=(tb == 15),
                            )
                    # normalize: hh=0 data on parts 0:64 (denom on 64:128);
                    # hh=1 mirrored.  Engines can't cross partitions, so the
                    # denominators bounce through SBUF via DMA.
                    rB = pl.tile([P, 512], F32, tag="rB", name="rB")
                    rC = pl.tile([P, 512], F32, tag="rC", name="rC")
                    if qc == 3 and p == 1:
                        # last iteration: nothing else needs the "out" slot,
                        # so skip the big PSUM->SBUF copy and run the
                        # shortest-latency chain straight off the accumulator
                        t0 = pl.tile([P, 512], F32, tag="t0", name="t0")
                        nc.vector.tensor_copy(t0[64:128, :], op[:, 0, :][64:128, :])
                        nc.scalar.activation(t0[0:64, :], op[:, 1, :][0:64, :],
                                             AF.Copy)
                        nc.sync.dma_start(rB[0:64, :], t0[64:128, :])
                        nc.sync.dma_start(rB[64:128, :], t0[0:64, :])
                        nc.vector.reciprocal(rC[:], rB[:])
                        nc.vector.tensor_tensor(
                            oT[0:64, p, qs], op[:, 0, :][0:64, :],
                            rC[0:64, :], OP.mult)
                        nc.vector.tensor_tensor(
                            oT[64:128, p, qs], op[:, 1, :][64:128, :],
                            rC[64:128, :], OP.mult)
                    else:
                        # steady state: one PSUM->SBUF copy releases the
                        # "out" slot fast; the rest of the chain runs from
                        # SBUF off the accumulator's critical path, with the
                        # scale ops on the otherwise-idle gpsimd engine
                        oU = pl.tile([P, 2, 512], F32, tag="oU", name="oU")
                        nc.vector.tensor_copy(oU[:], op[:])
                        nc.sync.dma_start(rB[0:64, :], oU[:, 0, :][64:128, :])
                        nc.sync.dma_start(rB[64:128, :], oU[:, 1, :][0:64, :])
                        nc.vector.reciprocal(rC[:], rB[:])
                        nc.gpsimd.tensor_tensor(
                            oT[0:64, p, qs], oU[:, 0, :][0:64, :],
                            rC[0:64, :], OP.mult)
                        nc.gpsimd.tensor_tensor(
                            oT[64:128, p, qs], oU[:, 1, :][64:128, :],
                            rC[64:128, :], OP.mult)

            # phase C for the last qc: the first-half (ch=0) matmuls depend
            # only on the p=0 rows of oT, so they run DURING the final
            # normalization chain and keep PE warm; the ch=1 halves follow
            # right after it.  Copies alternate between DVE and the now-idle
            # scalar engine; DMAs pipeline behind them.
            tags = ["b2", "qk0", "qk1", "out", "b2", "qk0", "qk1", "out"]
            pos = []
            for j in range(4):
                for jh in range(2):
                    qb = 12 + j
                    po = (psb if tags[2 * j + jh] == "b2" else ps).tile(
                        [P, 1, 512], F32, tag=tags[2 * j + jh], name="po")
                    nc.tensor.matmul(
                        po[:, 0, :],
                        lhsT=oT[:, 0, qb * P:(qb + 1) * P],
                        rhs=woT[:, 0, jh * 512:(jh + 1) * 512],
                        start=True, stop=False,
                    )
                    pos.append((po, j, jh, qb))
            for idx, (po, j, jh, qb) in enumerate(pos):
                nc.tensor.matmul(
                    po[:, 0, :],
                    lhsT=oT[:, 1, qb * P:(qb + 1) * P],
                    rhs=woT[:, 1, jh * 512:(jh + 1) * 512],
                    start=False, stop=True,
                )
                osb = plr.tile([P, 1, 512], BF16, tag="osb", name="osb")
                if idx % 2:
                    nc.scalar.activation(osb[:], po[:], AF.Copy)
                else:
                    nc.vector.tensor_copy(osb[:], po[:])
                nc.sync.dma_start(dout[:, qb, jh * 512:(jh + 1) * 512],
                                  osb[:, 0, :])

    nc.compile()
    return nc


def _prep_core_inputs(inputs, core):
    b, g = core // 4, core % 4
    C = slice(g * 256, (g + 1) * 256)
    Hx = np.asarray(inputs["Hx"], np.float32)
    Hf = np.asarray(inputs["Hf"], np.float32)
    Gm = np.asarray(inputs["Gm"], np.float32)
    Wg = np.asarray(inputs["Wg"], np.float32)
    bg = np.asarray(inputs["bg"], np.float32)
    Wq = np.asarray(inputs["Wq"], np.float32)
    bq = np.asarray(inputs["bq"], np.float32)
    Wk = np.asarray(inputs["Wk"], np.float32)
    bk = np.asarray(inputs["bk"], np.float32)
    Wv = np.asarray(inputs["Wv"], np.float32)
    bv = np.asarray(inputs["bv"], np.float32)
    Wo = np.asarray(inputs["Wo"], np.float32)

    bf = ml_dtypes.bfloat16
    s = 1.0 / 8.0  # 1/sqrt(DK) folded into Q
    return {
        "XTa": _pm_chunked(Hx[b, :, :, 0], bf),
        "XTb": _pm_chunked(Hf[b].T, bf),
        "WqaT": _pm(Wq[C, :1024].T * s, bf),
        "WqbT": _pm(Wq[C, 1024:].T * s, bf),
        "WkaT": _pm(Wk[C, :1024].T, bf),
        "WkbT": _pm(Wk[C, 1024:].T, bf),
        "WvT": _pm(Wv[C, :].T, bf),
        "WoT": _pm(Wo[:, C].T, bf),
        "GmT": _pm(Gm[b].T, ml_dtypes.float8_e4m3),
        "WgT": _pm(Wg.T, ml_dtypes.float8_e4m3),
        "smalls": np.ascontiguousarray(np.concatenate([
            (bq[C] * s).reshape(2, P).T,
            bk[C].reshape(2, P).T,
            bg.reshape(16, P).T,
            np.broadcast_to(bv[C], (P, 256)),
        ], axis=1, dtype=np.float32)),
    }


_NC_CACHE = []


def kernel(**inputs):
    if not _NC_CACHE:
        _NC_CACHE.append(build_nc())
    nc = _NC_CACHE[0]
    in_maps = [_prep_core_inputs(inputs, c) for c in range(8)]
    res = run_bass_kernel_spmd(nc, in_maps, core_ids=list(range(8)))
    bo = np.asarray(inputs["bo"], np.float32)
    out = np.zeros((B, T, D), np.float32)
    for b in range(B):
        acc = np.zeros((T, D), np.float32)
        for g in range(4):
            part = np.asarray(res.results[b * 4 + g]["outp"], np.float32)
            acc += part.transpose(1, 0, 2).reshape(T, D)
        out[b] = acc + bo[None, :]
    return out


# revision 59
# speedup vs baseline: 1.4014x; 1.0269x over previous
"""Bass/Tile TRN2 kernel for nn_BiasedMultiheadAttention (B=2,T=2048,D=1024,H=16,DM=256).

Sharding: 8 cores = batch(2) x head-group(4).  Each core computes 4 heads of one
batch element plus the (replicated) Gm bias linear, and emits an unnormalized
partial of the output projection; the host sums the 4 partials per batch and
adds bo.

v5 structure (phase B is ACT-paced at ~1.04us per key-block, so everything
that can leave phase B does):
  - all matmul operands bf16 (PSUM accumulation stays f32)
  - X loaded in column chunks [128, 8io, 256] so projection matmuls start
    after the first ~1.5us of DMA instead of after the full half
  - bg folded into the bias-exp via the activation bias operand
  - ALL bias GEMM+exp chains run pre-B where the scalar engine is idle;
    biasE for qc2/qc3 round-trips through DRAM scratch (SBUF holds two
    [128,16,512] biasE buffers)
  - single-bank PSUM tiles on a 2-slot pool ("b2") for V / bias / output
    projection / filler chains -> no long slot-serialization chains
  - ex multiply is one in-place broadcast DVE op per key block
  - output partials in bf16 (host sums in f32)
"""

import numpy as np
import ml_dtypes

import concourse.bass as bass
from concourse.bass import broadcast_tensor_aps
from concourse import bacc
import concourse.mybir as mybir
from concourse.tile import TileContext
from concourse.bass_utils import run_bass_kernel_spmd

B, T, D, H, DM = 2, 2048, 1024, 16, 256
P = 128
F32 = mybir.dt.float32
BF16 = mybir.dt.bfloat16
F8 = mybir.dt.float8e4
PM_DR = mybir.MatmulPerfMode.DoubleRow
AF = mybir.ActivationFunctionType
OP = mybir.AluOpType


def _pm(a, dt=np.float32):
    """(R, C) row-major -> partition-major (128, R//128, C), contiguous."""
    a = np.ascontiguousarray(a, dtype=np.float32)
    r, c = a.shape
    return np.ascontiguousarray(a.reshape(r // P, P, c).transpose(1, 0, 2).astype(dt))


def _pm_chunked(a, dt):
    """(R=1024, C=2048) -> (128, 4 col-chunks, 8 io-blocks, 512), so one
    chunk c holds columns [512c, 512c+512) of all 8 row-blocks."""
    x = _pm(a, dt)  # (128, 8, 2048)
    x = x.reshape(P, 8, 4, 512).transpose(0, 2, 1, 3)
    return np.ascontiguousarray(x)


def build_nc():
    nc = bacc.Bacc("TRN2", target_bir_lowering=False, debug=False)

    def inp(name, shape, dt=F32):
        return nc.dram_tensor(name, list(shape), dt, kind="ExternalInput")

    dXa = inp("XTa", (P, 4, 8, 512), BF16)
    dXb = inp("XTb", (P, 4, 8, 512), BF16)
    dWqa = inp("WqaT", (P, 8, 256), BF16)
    dWqb = inp("WqbT", (P, 8, 256), BF16)
    dWka = inp("WkaT", (P, 8, 256), BF16)
    dWkb = inp("WkbT", (P, 8, 256), BF16)
    dWv = inp("WvT", (P, 8, 256), BF16)
    dWo = inp("WoT", (P, 2, 1024), BF16)
    dGm = inp("GmT", (P, 2, 2048), F8)
    dWg = inp("WgT", (P, 2, 2048), F8)
    dsm = inp("smalls", (P, 276))  # [bq2 | bk2 | bgPM | bvb]
    dSw = inp("Sswap", (P, 128), mybir.dt.float32r)
    dout = nc.dram_tensor("outp", [P, 16, 1024], BF16, kind="ExternalOutput")

    with TileContext(nc) as tc:
        with tc.tile_pool(name="sb", bufs=1) as sb, \
             tc.tile_pool(name="ps", bufs=1, space="PSUM") as ps, \
             tc.tile_pool(name="psb", bufs=2, space="PSUM") as psb, \
             tc.tile_pool(name="pbe", bufs=2) as pbe, \
             tc.tile_pool(name="pstg", bufs=3) as pstg, \
             tc.tile_pool(name="pdr", bufs=1, space="DRAM") as pdr, \
             tc.tile_pool(name="plr", bufs=4) as plr, \
             tc.tile_pool(name="pl", bufs=1) as pl:
            QT = sb.tile([P, 2, 2048], BF16, tag="QT")
            KT = sb.tile([P, 2, 2048], BF16, tag="KT")
            Vt = sb.tile([P, 16, 512], BF16, tag="Vt")  # per tb: [V|1]/[1|V] x2
            oT = sb.tile([P, 2, 2048], BF16, tag="oT")
            woT = sb.tile([P, 2, 1024], BF16, tag="woT")
            gmT = sb.tile([P, 2, 2048], F8, tag="gmT")
            wgT = sb.tile([P, 2, 2048], F8, tag="wgT")
            sm = sb.tile([P, 276], F32, tag="sm")
            bq_s = sm[:, 0:2]
            bk_s = sm[:, 2:4]
            bg_s = sm[:, 4:20]
            bvb_s = sm[:, 20:276]
            wqa_s = sb.tile([P, 8, 256], BF16, tag="wqa")
            wqb_s = sb.tile([P, 8, 256], BF16, tag="wqb")
            wka_s = sb.tile([P, 8, 256], BF16, tag="wka")
            wkb_s = sb.tile([P, 8, 256], BF16, tag="wkb")
            wv_s = sb.tile([P, 8, 256], BF16, tag="wv")
            sw_s = sb.tile([P, 128], mybir.dt.float32r, tag="sw")
            xa = [sb.tile([P, 8, 512], BF16, tag=f"xa{c}", name=f"xa{c}")
                  for c in range(4)]
            xb = [sb.tile([P, 8, 512], BF16, tag=f"xb{c}", name=f"xb{c}")
                  for c in range(4)]
            dbE = {qc: pdr.tile([P, 16, 512], BF16, tag=f"dbE{qc}",
                                name=f"dbE{qc}")
                   for qc in (2, 3)}

            # --- input DMAs, in priority order; gm/wg chunked so the first
            # bias chains start ~5us in ---
            nc.sync.dma_start(sm[:], dsm[:])
            nc.sync.dma_start(sw_s[:], dSw[:])
            nc.vector.memset(
                Vt.rearrange("p t (a v) -> p (t a) v", a=2)[:, :, 64:192], 1.0)
            nc.sync.dma_start(gmT[:, :, 0:512], dGm[:, :, 0:512])
            nc.sync.dma_start(wgT[:, :, 0:1024], dWg[:, :, 0:1024])
            nc.sync.dma_start(wgT[:, :, 1024:2048], dWg[:, :, 1024:2048])
            nc.sync.dma_start(gmT[:, :, 512:2048], dGm[:, :, 512:2048])
            nc.sync.dma_start(xa[0][:], dXa[:, 0])
            nc.sync.dma_start(wka_s[:], dWka[:])
            nc.sync.dma_start(wqa_s[:], dWqa[:])
            nc.sync.dma_start(wv_s[:], dWv[:])
            for c in range(1, 4):
                nc.sync.dma_start(xa[c][:], dXa[:, c])
            for c in range(4):
                nc.sync.dma_start(xb[c][:], dXb[:, c])
            nc.sync.dma_start(wkb_s[:], dWkb[:])
            nc.sync.dma_start(wqb_s[:], dWqb[:])
            nc.sync.dma_start(woT[:], dWo[:])

            # ---------------- Phase A ----------------
            def emit_bias_tb(dst_ap, qct, tb):
                """dst_ap[:, :512] = exp(Wg Gm^T + bg) for key block tb vs
                query chunk qct.  fp8 DoubleRow folds both DM k-subtiles
                into one half-rate matmul (single-bank psum, 2-slot
                pipelining)."""
                tqs = slice(qct * 512, (qct + 1) * 512)
                bps = psb.tile([P, 1, 512], F32, tag="b2", name="bps")
                nc.tensor.matmul(
                    bps[:, 0, :],
                    lhsT=wgT[:, :, tb * P:(tb + 1) * P],
                    rhs=gmT[:, :, tqs],
                    start=True, stop=True,
                    perf_mode=PM_DR,
                )
                nc.scalar.activation(dst_ap, bps[:, 0, :], AF.Exp,
                                     bias=bg_s[:, tb:tb + 1])

            def emit_v(tb):
                """V chain for key block tb (hx only; xa chunk tb//4)."""
                vt = psb.tile([P, 1, 512], F32, tag="b2", name="vt")
                vps = vt[:, 0, 0:256]
                off = (tb % 4) * P
                for io in range(8):
                    nc.tensor.matmul(
                        vps,
                        lhsT=xa[tb // 4][:, io, off:off + P],
                        rhs=wv_s[:, io, :],
                        start=(io == 0), stop=(io == 7),
                    )
                # write data columns of Vt (+bv); ones from memset
                nc.vector.tensor_tensor(
                    Vt[:, tb].rearrange("p (a u v) -> p a u v", a=2, u=4)[:, :, 0:4:3, :],
                    vps.rearrange("p (a u v) -> p a u v", a=2, u=2),
                    bvb_s.rearrange("p (a u v) -> p a u v", a=2, u=2),
                    OP.add,
                )

            def emit_chain(xio, wt, dst, bias_s, qc, tag, first):
                """Full 2-plane projection chain on a 2-bank tag (phase A)."""
                qs = slice(qc * 512, (qc + 1) * 512)
                t = ps.tile([P, 2, 512], F32, tag=tag, name="t")
                for jb in range(2):
                    for io in range(8):
                        nc.tensor.matmul(
                            t[:, jb, :],
                            lhsT=wt[:, io, jb * P:(jb + 1) * P],
                            rhs=xio[qc][:, io, :],
                            start=(io == 0), stop=(io == 7),
                        )
                if first:
                    for jb in range(2):
                        nc.vector.tensor_scalar_add(
                            dst[:, jb, qs], t[:, jb, :], bias_s[:, jb:jb + 1])
                else:
                    nc.vector.tensor_tensor(
                        dst[:, :, qs], t[:], dst[:, :, qs], OP.add)

            def emit_chain_b2(xio, wt, dst, bias_s, qc, first):
                """Same chain split into per-jb single-bank tiles (B filler)."""
                qs = slice(qc * 512, (qc + 1) * 512)
                for jb in range(2):
                    t1 = psb.tile([P, 1, 512], F32, tag="b2", name="t1")
                    for io in range(8):
                        nc.tensor.matmul(
                            t1[:, 0, :],
                            lhsT=wt[:, io, jb * P:(jb + 1) * P],
                            rhs=xio[qc][:, io, :],
                            start=(io == 0), stop=(io == 7),
                        )
                    if first:
                        nc.vector.tensor_scalar_add(
                            dst[:, jb, qs], t1[:, 0, :], bias_s[:, jb:jb + 1])
                    else:
                        nc.vector.tensor_tensor(
                            dst[:, jb, qs], t1[:, 0, :], dst[:, jb, qs], OP.add)

            biasEs = {}
            biasEs[0] = pbe.tile([P, 16, 512], BF16, tag="biasE", name="biasE0")
            biasEs[1] = pbe.tile([P, 16, 512], BF16, tag="biasE", name="biasE1")
            for tb in range(16):
                emit_bias_tb(biasEs[0][:, tb, :], 0, tb)
            for tb in range(16):
                emit_bias_tb(biasEs[1][:, tb, :], 1, tb)
            for qct in (2, 3):
                for tg in range(4):
                    # 4 exps share one staging tile -> one DMA to DRAM
                    # scratch per group
                    stg = pstg.tile([P, 4, 512], BF16, tag="bstg", name="bstg")
                    for ti in range(4):
                        emit_bias_tb(stg[:, ti, :], qct, 4 * tg + ti)
                    nc.sync.dma_start(dbE[qct][:, 4 * tg:4 * tg + 4, :], stg[:])
            for qc in range(4):
                emit_chain(xa, wka_s, KT, bk_s, qc, "qk1", True)
            emit_chain(xa, wqa_s, QT, bq_s, 0, "qk0", True)
            for qc in range(4):
                emit_chain(xb, wkb_s, KT, bk_s, qc, "qk1", False)
            emit_chain(xb, wqb_s, QT, bq_s, 0, "qk0", False)
            for tb in range(6):
                emit_v(tb)

            # ---------------- Phase B + C ----------------
            def emit_norm(op, qs, p, last):
                """Normalize op into oT[:, p, qs].  hh=0 data on parts 0:64
                (denom replicated on 64:128); hh=1 mirrored.  The
                cross-partition denominator swap is a 213ns permutation
                matmul.  Called DEFERRED (mid next block) so the PE-stream
                entry never stalls on the copies feeding it."""
                t0 = pl.tile([P, 512], mybir.dt.float32r, tag="t0", name="t0")
                rBp = psb.tile([P, 1, 512], F32, tag="b2", name="rBp")
                rC = pl.tile([P, 512], F32, tag="rC", name="rC")
                if last:
                    nc.vector.tensor_copy(t0[64:128, :], op[:, 0, :][64:128, :])
                    nc.scalar.activation(t0[0:64, :], op[:, 1, :][0:64, :],
                                         AF.Copy)
                    nc.tensor.matmul(rBp[:, 0, :], lhsT=sw_s[:],
                                     rhs=t0[:], start=True, stop=True)
                    nc.vector.reciprocal(rC[:], rBp[:, 0, :])
                    nc.vector.tensor_tensor(
                        oT[0:64, p, qs], op[:, 0, :][0:64, :],
                        rC[0:64, :], OP.mult)
                    nc.vector.tensor_tensor(
                        oT[64:128, p, qs], op[:, 1, :][64:128, :],
                        rC[64:128, :], OP.mult)
                else:
                    # one PSUM->SBUF copy releases the "out" slot fast; the
                    # rest runs off the critical path with the scale ops on
                    # the otherwise-idle gpsimd engine
                    oU = pl.tile([P, 2, 512], F32, tag="oU", name="oU")
                    nc.vector.tensor_copy(oU[:], op[:])
                    nc.gpsimd.tensor_copy(t0[64:128, :], oU[:, 0, :][64:128, :])
                    nc.gpsimd.tensor_copy(t0[0:64, :], oU[:, 1, :][0:64, :])
                    nc.tensor.matmul(rBp[:, 0, :], lhsT=sw_s[:],
                                     rhs=t0[:], start=True, stop=True)
                    nc.vector.reciprocal(rC[:], rBp[:, 0, :])
                    nc.gpsimd.tensor_tensor(
                        oT[0:64, p, qs], oU[:, 0, :][0:64, :],
                        rC[0:64, :], OP.mult)
                    nc.gpsimd.tensor_tensor(
                        oT[64:128, p, qs], oU[:, 1, :][64:128, :],
                        rC[64:128, :], OP.mult)

            osbjs = {}

            def emit_c(qct, j, jh, tag="b2"):
                """Output projection for query block qb, half jh.  Both
                halves share one osb tile and one DMA (issued at jh=1) so
                output writes don't queue-jam the DMA engine in front of
                the normalization bounces."""
                qb = 4 * qct + j
                if tag == "b2":
                    po = psb.tile([P, 1, 512], F32, tag=tag, name="po")
                else:
                    po = ps.tile([P, 1, 512], F32, tag=tag, name="po")
                for ch in range(2):
                    nc.tensor.matmul(
                        po[:, 0, :],
                        lhsT=oT[:, ch, qb * P:(qb + 1) * P],
                        rhs=woT[:, ch, jh * 512:(jh + 1) * 512],
                        start=(ch == 0), stop=(ch == 1),
                    )
                if jh == 0:
                    osbjs[qb] = plr.tile([P, 1024], BF16, tag="osb",
                                         name="osb")
                osb = osbjs[qb]
                nc.vector.tensor_copy(osb[:, jh * 512:(jh + 1) * 512],
                                      po[:, 0, :])
                if jh == 1:
                    nc.sync.dma_start(dout[:, qb, :], osbjs.pop(qb)[:])

            pending = None
            for qc in range(4):
                qs = slice(qc * 512, (qc + 1) * 512)
                if qc >= 2:
                    # reload this qc's biasE from DRAM into a freed SBUF slot
                    # (chunked so normalization bounce DMAs don't queue
                    # behind one long transfer)
                    bE = pbe.tile([P, 16, 512], BF16, tag="biasE",
                                  name=f"biasE{qc}")
                    for ck in range(4):
                        nc.sync.dma_start(bE[:, 4 * ck:4 * ck + 4, :],
                                          dbE[qc][:, 4 * ck:4 * ck + 4, :])
                    biasEs[qc] = bE
                biasE = biasEs[qc]
                for p in range(2):
                    op = ps.tile([P, 2, 512], F32, tag="out", name="op")
                    for tb in range(16):
                        qk = ps.tile([P, 2, 512], F32, tag=f"qk{tb % 2}",
                                     name=f"qk{tb % 2}")
                        for hh in range(2):
                            dd = slice(hh * 64, (hh + 1) * 64)
                            nc.tensor.matmul(
                                qk[:, hh, :],
                                lhsT=KT[dd, p, tb * P:(tb + 1) * P],
                                rhs=QT[dd, p, qs],
                                start=True, stop=True,
                            )
                        # fillers that keep PE fed under the ACT exp pace:
                        # remaining V chains (qc0), the next qc's Q chains,
                        # and the previous qc's output projections
                        if qc == 0 and p == 0 and tb < 10:
                            emit_v(tb + 6)
                        if p == 0 and tb == 2 and qc < 3:
                            emit_chain_b2(xa, wqa_s, QT, bq_s, qc + 1, True)
                        if p == 0 and tb == 12 and qc < 3:
                            emit_chain_b2(xb, wqb_s, QT, bq_s, qc + 1, False)
                        if tb == 6 and pending is not None:
                            emit_norm(*pending)
                            pending = None
                        ex1 = plr.tile([P, 2, 512], BF16, tag="ex1", name="ex1")
                        nc.scalar.activation(ex1[:], qk[:], AF.Exp)
                        eb_a, eb_b = broadcast_tensor_aps(
                            ex1[:], biasE[:, tb, :].rearrange(
                                "p (a q) -> p a q", a=1))
                        nc.vector.tensor_tensor(ex1[:], eb_a, eb_b, OP.mult)
                        if p == 1 and tb % 4 == 0 and qc > 0:
                            emit_c(qc - 1, tb // 4, 0)
                        if p == 1 and tb % 4 == 2 and qc > 0:
                            emit_c(qc - 1, tb // 4, 1)
                        for hh in range(2):
                            h = 2 * p + hh
                            nc.tensor.matmul(
                                op[:, hh, :],
                                lhsT=Vt[:, tb, h * P:(h + 1) * P],
                                rhs=ex1[:, hh, :],
                                start=(tb == 0), stop=(tb == 15),
                            )
                    pending = (op, qs, p, False)

            op_f, qs_f, p_f, _ = pending
            emit_norm(op_f, qs_f, p_f, True)

            # phase C for the last qc: the first-half (ch=0) matmuls depend
            # only on the p=0 rows of oT, so they run DURING the final
            # normalization chain and keep PE warm; the ch=1 halves follow
            # right after it.  Copies alternate between DVE and the now-idle
            # scalar engine; DMAs pipeline behind them.
            tags = ["b2", "qk0", "qk1", "out", "b2", "qk0", "qk1", "out"]
            pos = []
            for j in range(4):
                for jh in range(2):
                    qb = 12 + j
                    po = (psb if tags[2 * j + jh] == "b2" else ps).tile(
                        [P, 1, 512], F32, tag=tags[2 * j + jh], name="po")
                    nc.tensor.matmul(
                        po[:, 0, :],
                        lhsT=oT[:, 0, qb * P:(qb + 1) * P],
                        rhs=woT[:, 0, jh * 512:(jh + 1) * 512],
                        start=True, stop=False,
                    )
                    pos.append((po, j, jh, qb))
            for idx, (po, j, jh, qb) in enumerate(pos):
                nc.tensor.matmul(
                    po[:, 0, :],
                    lhsT=oT[:, 1, qb * P:(qb + 1) * P],
                    rhs=woT[:, 1, jh * 512:(jh + 1) * 512],
                    start=False, stop=True,
                )
                if jh == 0:
                    osbjs[qb] = plr.tile([P, 1024], BF16, tag="osb",
                                         name="osb")
                dst = osbjs[qb][:, jh * 512:(jh + 1) * 512]
                if idx % 2:
                    nc.scalar.activation(dst, po[:, 0, :], AF.Copy)
                else:
                    nc.vector.tensor_copy(dst, po[:, 0, :])
                if jh == 1:
                    nc.sync.dma_start(dout[:, qb, :], osbjs.pop(qb)[:])

    nc.compile()
    return nc


def _prep_core_inputs(inputs, core):
    b, g = core // 4, core % 4
    C = slice(g * 256, (g + 1) * 256)
    Hx = np.asarray(inputs["Hx"], np.float32)
    Hf = np.asarray(inputs["Hf"], np.float32)
    Gm = np.asarray(inputs["Gm"], np.float32)
    Wg = np.asarray(inputs["Wg"], np.float32)
    bg = np.asarray(inputs["bg"], np.float32)
    Wq = np.asarray(inputs["Wq"], np.float32)
    bq = np.asarray(inputs["bq"], np.float32)
    Wk = np.asarray(inputs["Wk"], np.float32)
    bk = np.asarray(inputs["bk"], np.float32)
    Wv = np.asarray(inputs["Wv"], np.float32)
    bv = np.asarray(inputs["bv"], np.float32)
    Wo = np.asarray(inputs["Wo"], np.float32)

    bf = ml_dtypes.bfloat16
    s = 1.0 / 8.0  # 1/sqrt(DK) folded into Q
    return {
        "XTa": _pm_chunked(Hx[b, :, :, 0], bf),
        "XTb": _pm_chunked(Hf[b].T, bf),
        "WqaT": _pm(Wq[C, :1024].T * s, bf),
        "WqbT": _pm(Wq[C, 1024:].T * s, bf),
        "WkaT": _pm(Wk[C, :1024].T, bf),
        "WkbT": _pm(Wk[C, 1024:].T, bf),
        "WvT": _pm(Wv[C, :].T, bf),
        "WoT": _pm(Wo[:, C].T, bf),
        "GmT": _pm(Gm[b].T, ml_dtypes.float8_e4m3),
        "WgT": _pm(Wg.T, ml_dtypes.float8_e4m3),
        "Sswap": np.ascontiguousarray(np.roll(np.eye(P, dtype=np.float32),
                                              64, axis=0)),
        "smalls": np.ascontiguousarray(np.concatenate([
            (bq[C] * s).reshape(2, P).T,
            bk[C].reshape(2, P).T,
            bg.reshape(16, P).T,
            np.broadcast_to(bv[C], (P, 256)),
        ], axis=1, dtype=np.float32)),
    }


_NC_CACHE = []


def kernel(**inputs):
    if not _NC_CACHE:
        _NC_CACHE.append(build_nc())
    nc = _NC_CACHE[0]
    in_maps = [_prep_core_inputs(inputs, c) for c in range(8)]
    res = run_bass_kernel_spmd(nc, in_maps, core_ids=list(range(8)))
    bo = np.asarray(inputs["bo"], np.float32)
    out = np.zeros((B, T, D), np.float32)
    for b in range(B):
        acc = np.zeros((T, D), np.float32)
        for g in range(4):
            part = np.asarray(res.results[b * 4 + g]["outp"], np.float32)
            acc += part.transpose(1, 0, 2).reshape(T, D)
        out[b] = acc + bo[None, :]
    return out


# revision 76
# speedup vs baseline: 1.4790x; 1.0554x over previous
"""Bass/Tile TRN2 kernel for nn_BiasedMultiheadAttention (B=2,T=2048,D=1024,H=16,DM=256).

Sharding: 8 cores = batch(2) x head-group(4).  Each core computes 4 heads of one
batch element plus the (replicated) Gm bias linear, and emits an unnormalized
partial of the output projection; the host sums the 4 partials per batch and
adds bo.

v5 structure (phase B is ACT-paced at ~1.04us per key-block, so everything
that can leave phase B does):
  - all matmul operands bf16 (PSUM accumulation stays f32)
  - X loaded in column chunks [128, 8io, 256] so projection matmuls start
    after the first ~1.5us of DMA instead of after the full half
  - bg folded into the bias-exp via the activation bias operand
  - ALL bias GEMM+exp chains run pre-B where the scalar engine is idle;
    biasE for qc2/qc3 round-trips through DRAM scratch (SBUF holds two
    [128,16,512] biasE buffers)
  - single-bank PSUM tiles on a 2-slot pool ("b2") for V / bias / output
    projection / filler chains -> no long slot-serialization chains
  - ex multiply is one in-place broadcast DVE op per key block
  - output partials in bf16 (host sums in f32)
"""

import numpy as np
import ml_dtypes

import concourse.bass as bass
from concourse.bass import broadcast_tensor_aps
from concourse import bacc
import concourse.mybir as mybir
from concourse.tile import TileContext
from concourse.bass_utils import run_bass_kernel_spmd

B, T, D, H, DM = 2, 2048, 1024, 16, 256
P = 128
F32 = mybir.dt.float32
BF16 = mybir.dt.bfloat16
F8 = mybir.dt.float8e4
PM_DR = mybir.MatmulPerfMode.DoubleRow
AF = mybir.ActivationFunctionType
OP = mybir.AluOpType


def _pm(a, dt=np.float32):
    """(R, C) row-major -> partition-major (128, R//128, C), contiguous."""
    a = np.ascontiguousarray(a, dtype=np.float32)
    r, c = a.shape
    return np.ascontiguousarray(a.reshape(r // P, P, c).transpose(1, 0, 2).astype(dt))


def _pm_chunked(a, dt):
    """(R=1024, C=2048) -> (128, 4 col-chunks, 8 io-blocks, 512), so one
    chunk c holds columns [512c, 512c+512) of all 8 row-blocks."""
    x = _pm(a, dt)  # (128, 8, 2048)
    x = x.reshape(P, 8, 4, 512).transpose(0, 2, 1, 3)
    return np.ascontiguousarray(x)


def build_nc():
    nc = bacc.Bacc("TRN2", target_bir_lowering=False, debug=False)

    def inp(name, shape, dt=F32):
        return nc.dram_tensor(name, list(shape), dt, kind="ExternalInput")

    dXa = inp("XTa", (P, 4, 8, 512), BF16)
    dXb = inp("XTb", (P, 4, 8, 512), BF16)
    dWqa = inp("WqaT", (P, 8, 256), BF16)
    dWqb = inp("WqbT", (P, 8, 256), BF16)
    dWka = inp("WkaT", (P, 8, 256), BF16)
    dWkb = inp("WkbT", (P, 8, 256), BF16)
    dWv = inp("WvT", (P, 8, 256), BF16)
    dWo = inp("WoT", (P, 2, 1024), BF16)
    dGm = inp("GmT", (P, 2, 2048), F8)
    dWg = inp("WgT", (P, 2, 2048), F8)
    dsm = inp("smalls", (P, 276))  # [bq2 | bk2 | bgPM | bvb]
    dSw = inp("Sswap", (P, 128), mybir.dt.float32r)
    dout = nc.dram_tensor("outp", [P, 16, 1024], BF16, kind="ExternalOutput")

    with TileContext(nc) as tc:
        with tc.tile_pool(name="sb", bufs=1) as sb, \
             tc.tile_pool(name="ps", bufs=1, space="PSUM") as ps, \
             tc.tile_pool(name="psb", bufs=2, space="PSUM") as psb, \
             tc.tile_pool(name="pbe", bufs=2) as pbe, \
             tc.tile_pool(name="pstg", bufs=3) as pstg, \
             tc.tile_pool(name="pdr", bufs=1, space="DRAM") as pdr, \
             tc.tile_pool(name="plr", bufs=4) as plr, \
             tc.tile_pool(name="pl", bufs=1) as pl:
            QT = sb.tile([P, 2, 2048], BF16, tag="QT")
            KT = sb.tile([P, 2, 2048], BF16, tag="KT")
            Vt = sb.tile([P, 16, 512], BF16, tag="Vt")  # per tb: [V|1]/[1|V] x2
            oT = sb.tile([P, 2, 2048], BF16, tag="oT")
            woT = sb.tile([P, 2, 1024], BF16, tag="woT")
            gmT = sb.tile([P, 2, 2048], F8, tag="gmT")
            wgT = sb.tile([P, 2, 2048], F8, tag="wgT")
            sm = sb.tile([P, 276], F32, tag="sm")
            bq_s = sm[:, 0:2]
            bk_s = sm[:, 2:4]
            bg_s = sm[:, 4:20]
            bvb_s = sm[:, 20:276]
            wqa_s = sb.tile([P, 8, 256], BF16, tag="wqa")
            wqb_s = sb.tile([P, 8, 256], BF16, tag="wqb")
            wka_s = sb.tile([P, 8, 256], BF16, tag="wka")
            wkb_s = sb.tile([P, 8, 256], BF16, tag="wkb")
            wv_s = sb.tile([P, 8, 256], BF16, tag="wv")
            sw_s = sb.tile([P, 128], mybir.dt.float32r, tag="sw")
            xa = [sb.tile([P, 8, 512], BF16, tag=f"xa{c}", name=f"xa{c}")
                  for c in range(4)]
            xb = [sb.tile([P, 8, 512], BF16, tag=f"xb{c}", name=f"xb{c}")
                  for c in range(4)]
            dbE = {qc: pdr.tile([P, 16, 512], BF16, tag=f"dbE{qc}",
                                name=f"dbE{qc}")
                   for qc in (2, 3)}

            # --- input DMAs, in priority order; gm/wg chunked so the first
            # bias chains start ~5us in ---
            nc.sync.dma_start(sm[:], dsm[:])
            nc.sync.dma_start(sw_s[:], dSw[:])
            nc.vector.memset(
                Vt.rearrange("p t (a v) -> p (t a) v", a=2)[:, :, 64:192], 1.0)
            nc.sync.dma_start(gmT[:, :, 0:512], dGm[:, :, 0:512])
            nc.sync.dma_start(wgT[:, :, 0:1024], dWg[:, :, 0:1024])
            nc.sync.dma_start(wgT[:, :, 1024:2048], dWg[:, :, 1024:2048])
            nc.sync.dma_start(gmT[:, :, 512:2048], dGm[:, :, 512:2048])
            nc.sync.dma_start(xa[0][:], dXa[:, 0])
            nc.sync.dma_start(wka_s[:], dWka[:])
            nc.sync.dma_start(wqa_s[:], dWqa[:])
            nc.sync.dma_start(wv_s[:], dWv[:])
            for c in range(1, 4):
                nc.sync.dma_start(xa[c][:], dXa[:, c])
            nc.sync.dma_start(wkb_s[:], dWkb[:])
            nc.sync.dma_start(wqb_s[:], dWqb[:])
            for c in range(4):
                nc.sync.dma_start(xb[c][:], dXb[:, c])
            nc.sync.dma_start(woT[:], dWo[:])

            # ---------------- Phase A ----------------
            def emit_bias_tb(dst_ap, qct, tb):
                """dst_ap[:, :512] = exp(Wg Gm^T + bg) for key block tb vs
                query chunk qct.  fp8 DoubleRow folds both DM k-subtiles
                into one half-rate matmul (single-bank psum, 2-slot
                pipelining)."""
                tqs = slice(qct * 512, (qct + 1) * 512)
                bps = psb.tile([P, 1, 512], F32, tag="b2", name="bps")
                nc.tensor.matmul(
                    bps[:, 0, :],
                    lhsT=wgT[:, :, tb * P:(tb + 1) * P],
                    rhs=gmT[:, :, tqs],
                    start=True, stop=True,
                    perf_mode=PM_DR,
                )
                nc.scalar.activation(dst_ap, bps[:, 0, :], AF.Exp,
                                     bias=bg_s[:, tb:tb + 1])

            def emit_v(tb):
                """V chain for key block tb (hx only; xa chunk tb//4)."""
                vt = psb.tile([P, 1, 512], F32, tag="b2", name="vt")
                vps = vt[:, 0, 0:256]
                off = (tb % 4) * P
                for io in range(8):
                    nc.tensor.matmul(
                        vps,
                        lhsT=xa[tb // 4][:, io, off:off + P],
                        rhs=wv_s[:, io, :],
                        start=(io == 0), stop=(io == 7),
                    )
                # write data columns of Vt (+bv); ones from memset
                nc.vector.tensor_tensor(
                    Vt[:, tb].rearrange("p (a u v) -> p a u v", a=2, u=4)[:, :, 0:4:3, :],
                    vps.rearrange("p (a u v) -> p a u v", a=2, u=2),
                    bvb_s.rearrange("p (a u v) -> p a u v", a=2, u=2),
                    OP.add,
                )

            def emit_chain(xio, wt, dst, bias_s, qc, tag, first):
                """Full 2-plane projection chain on a 2-bank tag (phase A)."""
                qs = slice(qc * 512, (qc + 1) * 512)
                t = ps.tile([P, 2, 512], F32, tag=tag, name="t")
                for jb in range(2):
                    for io in range(8):
                        nc.tensor.matmul(
                            t[:, jb, :],
                            lhsT=wt[:, io, jb * P:(jb + 1) * P],
                            rhs=xio[qc][:, io, :],
                            start=(io == 0), stop=(io == 7),
                        )
                if first:
                    for jb in range(2):
                        nc.vector.tensor_scalar_add(
                            dst[:, jb, qs], t[:, jb, :], bias_s[:, jb:jb + 1])
                else:
                    nc.vector.tensor_tensor(
                        dst[:, :, qs], t[:], dst[:, :, qs], OP.add)

            def emit_chain_b2(xio, wt, dst, bias_s, qc, first):
                """Same chain split into per-jb single-bank tiles (B filler)."""
                qs = slice(qc * 512, (qc + 1) * 512)
                for jb in range(2):
                    t1 = psb.tile([P, 1, 512], F32, tag="b2", name="t1")
                    for io in range(8):
                        nc.tensor.matmul(
                            t1[:, 0, :],
                            lhsT=wt[:, io, jb * P:(jb + 1) * P],
                            rhs=xio[qc][:, io, :],
                            start=(io == 0), stop=(io == 7),
                        )
                    if first:
                        nc.vector.tensor_scalar_add(
                            dst[:, jb, qs], t1[:, 0, :], bias_s[:, jb:jb + 1])
                    else:
                        nc.vector.tensor_tensor(
                            dst[:, jb, qs], t1[:, 0, :], dst[:, jb, qs], OP.add)

            biasEs = {}
            biasEs[0] = pbe.tile([P, 16, 512], BF16, tag="biasE", name="biasE0")
            biasEs[1] = pbe.tile([P, 16, 512], BF16, tag="biasE", name="biasE1")
            for tb in range(16):
                emit_bias_tb(biasEs[0][:, tb, :], 0, tb)
            for tb in range(16):
                emit_bias_tb(biasEs[1][:, tb, :], 1, tb)
            for qct in (2, 3):
                for tg in range(4):
                    # 4 exps share one staging tile -> one DMA to DRAM
                    # scratch per group
                    stg = pstg.tile([P, 4, 512], BF16, tag="bstg", name="bstg")
                    for ti in range(4):
                        emit_bias_tb(stg[:, ti, :], qct, 4 * tg + ti)
                    nc.sync.dma_start(dbE[qct][:, 4 * tg:4 * tg + 4, :], stg[:])
            for qc in range(4):
                emit_chain(xa, wka_s, KT, bk_s, qc, "qk1", True)
            emit_chain(xa, wqa_s, QT, bq_s, 0, "qk0", True)
            for qc in range(4):
                emit_chain(xb, wkb_s, KT, bk_s, qc, "qk1", False)
            emit_chain(xb, wqb_s, QT, bq_s, 0, "qk0", False)
            for tb in range(16):
                emit_v(tb)

            # ---------------- Phase B + C ----------------
            def emit_norm(op, qs, p, last):
                """Normalize op into oT[:, p, qs].  hh=0 data on parts 0:64
                (denom replicated on 64:128); hh=1 mirrored.  The
                cross-partition denominator swap is a 213ns permutation
                matmul.  Called DEFERRED (mid next block) so the PE-stream
                entry never stalls on the copies feeding it."""
                t0 = pl.tile([P, 512], mybir.dt.float32r, tag="t0", name="t0")
                rBp = psb.tile([P, 1, 512], F32, tag="b2", name="rBp")
                rC = pl.tile([P, 512], F32, tag="rC", name="rC")
                if last:
                    nc.vector.tensor_copy(t0[64:128, :], op[:, 0, :][64:128, :])
                    nc.scalar.activation(t0[0:64, :], op[:, 1, :][0:64, :],
                                         AF.Copy)
                    nc.tensor.matmul(rBp[:, 0, :], lhsT=sw_s[:],
                                     rhs=t0[:], start=True, stop=True)
                    nc.vector.reciprocal(rC[:], rBp[:, 0, :])
                    nc.vector.tensor_tensor(
                        oT[0:64, p, qs], op[:, 0, :][0:64, :],
                        rC[0:64, :], OP.mult)
                    nc.vector.tensor_tensor(
                        oT[64:128, p, qs], op[:, 1, :][64:128, :],
                        rC[64:128, :], OP.mult)
                else:
                    oU = op
                    nc.gpsimd.tensor_copy(t0[64:128, :], oU[:, 0, :][64:128, :])
                    nc.gpsimd.tensor_copy(t0[0:64, :], oU[:, 1, :][0:64, :])
                    nc.tensor.matmul(rBp[:, 0, :], lhsT=sw_s[:],
                                     rhs=t0[:], start=True, stop=True)
                    nc.vector.reciprocal(rC[:], rBp[:, 0, :])
                    nc.gpsimd.tensor_tensor(
                        oT[0:64, p, qs], oU[:, 0, :][0:64, :],
                        rC[0:64, :], OP.mult)
                    nc.gpsimd.tensor_tensor(
                        oT[64:128, p, qs], oU[:, 1, :][64:128, :],
                        rC[64:128, :], OP.mult)

            osbjs = {}

            def emit_c(qct, j, jh, tag="b2"):
                """Output projection for query block qb, half jh.  Both
                halves share one osb tile and one DMA (issued at jh=1) so
                output writes don't queue-jam the DMA engine in front of
                the normalization bounces."""
                qb = 4 * qct + j
                if tag == "b2":
                    po = psb.tile([P, 1, 512], F32, tag=tag, name="po")
                else:
                    po = ps.tile([P, 1, 512], F32, tag=tag, name="po")
                for ch in range(2):
                    nc.tensor.matmul(
                        po[:, 0, :],
                        lhsT=oT[:, ch, qb * P:(qb + 1) * P],
                        rhs=woT[:, ch, jh * 512:(jh + 1) * 512],
                        start=(ch == 0), stop=(ch == 1),
                    )
                if jh == 0:
                    osbjs[qb] = plr.tile([P, 1024], BF16, tag="osb",
                                         name="osb")
                osb = osbjs[qb]
                nc.vector.tensor_copy(osb[:, jh * 512:(jh + 1) * 512],
                                      po[:, 0, :])
                if jh == 1:
                    nc.sync.dma_start(dout[:, qb, :], osbjs.pop(qb)[:])

            pending = None
            for qc in range(4):
                qs = slice(qc * 512, (qc + 1) * 512)
                if qc >= 2:
                    # reload this qc's biasE from DRAM into a freed SBUF slot
                    # (chunked so normalization bounce DMAs don't queue
                    # behind one long transfer)
                    bE = pbe.tile([P, 16, 512], BF16, tag="biasE",
                                  name=f"biasE{qc}")
                    for ck in range(4):
                        nc.sync.dma_start(bE[:, 4 * ck:4 * ck + 4, :],
                                          dbE[qc][:, 4 * ck:4 * ck + 4, :])
                    biasEs[qc] = bE
                biasE = biasEs[qc]
                for p in range(2):
                    op = ps.tile([P, 2, 512], F32, tag="out", name="op")
                    for tb in range(16):
                        qk = ps.tile([P, 2, 512], F32, tag=f"qk{tb % 2}",
                                     name=f"qk{tb % 2}")
                        for hh in range(2):
                            dd = slice(hh * 64, (hh + 1) * 64)
                            nc.tensor.matmul(
                                qk[:, hh, :],
                                lhsT=KT[dd, p, tb * P:(tb + 1) * P],
                                rhs=QT[dd, p, qs],
                                start=True, stop=True,
                            )
                        # fillers that keep PE fed under the ACT exp pace:
                        # remaining V chains (qc0), the next qc's Q chains,
                        # and the previous qc's output projections
                        if p == 0 and tb == 4 and qc < 3:
                            emit_chain_b2(xa, wqa_s, QT, bq_s, qc + 1, True)
                        if p == 0 and tb == 13 and qc < 3:
                            emit_chain_b2(xb, wqb_s, QT, bq_s, qc + 1, False)
                        if tb == 5 and pending is not None:
                            emit_norm(*pending)
                            pending = None
                        ex1 = plr.tile([P, 2, 512], BF16, tag="ex1", name="ex1")
                        nc.scalar.activation(ex1[:], qk[:], AF.Exp)
                        eb_a, eb_b = broadcast_tensor_aps(
                            ex1[:], biasE[:, tb, :].rearrange(
                                "p (a q) -> p a q", a=1))
                        nc.vector.tensor_tensor(ex1[:], eb_a, eb_b, OP.mult)
                        if p == 1 and tb % 4 == 0 and qc > 0:
                            emit_c(qc - 1, tb // 4, 0)
                        if p == 1 and tb % 4 == 2 and qc > 0:
                            emit_c(qc - 1, tb // 4, 1)
                        for hh in range(2):
                            h = 2 * p + hh
                            nc.tensor.matmul(
                                op[:, hh, :],
                                lhsT=Vt[:, tb, h * P:(h + 1) * P],
                                rhs=ex1[:, hh, :],
                                start=(tb == 0), stop=(tb == 15),
                            )
                    # stage 1 of the deferred normalization: one
                    # PSUM->SBUF copy releases the "out" slot fast; the rest
                    # of the chain is emitted mid next block (emit_norm)
                    oU0 = pl.tile([P, 2, 512], F32, tag="oU", name="oU0")
                    nc.vector.tensor_copy(oU0[:], op[:])
                    pending = (oU0, qs, p, False)

            op_f, qs_f, p_f, _ = pending
            emit_norm(op_f, qs_f, p_f, True)

            # phase C for the last qc: the first-half (ch=0) matmuls depend
            # only on the p=0 rows of oT, so they run DURING the final
            # normalization chain and keep PE warm; the ch=1 halves follow
            # right after it.  Copies alternate between DVE and the now-idle
            # scalar engine; DMAs pipeline behind them.
            tags = ["b2", "qk0", "qk1", "out", "b2", "qk0", "qk1", "out"]
            pos = []
            for j in range(4):
                for jh in range(2):
                    qb = 12 + j
                    po = (psb if tags[2 * j + jh] == "b2" else ps).tile(
                        [P, 1, 512], F32, tag=tags[2 * j + jh], name="po")
                    nc.tensor.matmul(
                        po[:, 0, :],
                        lhsT=oT[:, 0, qb * P:(qb + 1) * P],
                        rhs=woT[:, 0, jh * 512:(jh + 1) * 512],
                        start=True, stop=False,
                    )
                    pos.append((po, j, jh, qb))
            for idx, (po, j, jh, qb) in enumerate(pos):
                nc.tensor.matmul(
                    po[:, 0, :],
                    lhsT=oT[:, 1, qb * P:(qb + 1) * P],
                    rhs=woT[:, 1, jh * 512:(jh + 1) * 512],
                    start=False, stop=True,
                )
                if jh == 0:
                    osbjs[qb] = plr.tile([P, 1024], BF16, tag="osb",
                                         name="osb")
                dst = osbjs[qb][:, jh * 512:(jh + 1) * 512]
                if idx % 2:
                    nc.scalar.activation(dst, po[:, 0, :], AF.Copy)
                else:
                    nc.vector.tensor_copy(dst, po[:, 0, :])
                if jh == 1:
                    nc.sync.dma_start(dout[:, qb, :], osbjs.pop(qb)[:])

    nc.compile()
    return nc


def _prep_core_inputs(inputs, core):
    b, g = core // 4, core % 4
    C = slice(g * 256, (g + 1) * 256)
    Hx = np.asarray(inputs["Hx"], np.float32)
    Hf = np.asarray(inputs["Hf"], np.float32)
    Gm = np.asarray(inputs["Gm"], np.float32)
    Wg = np.asarray(inputs["Wg"], np.float32)
    bg = np.asarray(inputs["bg"], np.float32)
    Wq = np.asarray(inputs["Wq"], np.float32)
    bq = np.asarray(inputs["bq"], np.float32)
    Wk = np.asarray(inputs["Wk"], np.float32)
    bk = np.asarray(inputs["bk"], np.float32)
    Wv = np.asarray(inputs["Wv"], np.float32)
    bv = np.asarray(inputs["bv"], np.float32)
    Wo = np.asarray(inputs["Wo"], np.float32)

    bf = ml_dtypes.bfloat16
    s = 1.0 / 8.0  # 1/sqrt(DK) folded into Q
    return {
        "XTa": _pm_chunked(Hx[b, :, :, 0], bf),
        "XTb": _pm_chunked(Hf[b].T, bf),
        "WqaT": _pm(Wq[C, :1024].T * s, bf),
        "WqbT": _pm(Wq[C, 1024:].T * s, bf),
        "WkaT": _pm(Wk[C, :1024].T, bf),
        "WkbT": _pm(Wk[C, 1024:].T, bf),
        "WvT": _pm(Wv[C, :].T, bf),
        "WoT": _pm(Wo[:, C].T, bf),
        "GmT": _pm(Gm[b].T, ml_dtypes.float8_e4m3),
        "WgT": _pm(Wg.T, ml_dtypes.float8_e4m3),
        "Sswap": np.ascontiguousarray(np.roll(np.eye(P, dtype=np.float32),
                                              64, axis=0)),
        "smalls": np.ascontiguousarray(np.concatenate([
            (bq[C] * s).reshape(2, P).T,
            bk[C].reshape(2, P).T,
            bg.reshape(16, P).T,
            np.broadcast_to(bv[C], (P, 256)),
        ], axis=1, dtype=np.float32)),
    }


_NC_CACHE = []


def kernel(**inputs):
    if not _NC_CACHE:
        _NC_CACHE.append(build_nc())
    nc = _NC_CACHE[0]
    in_maps = [_prep_core_inputs(inputs, c) for c in range(8)]
    res = run_bass_kernel_spmd(nc, in_maps, core_ids=list(range(8)))
    bo = np.asarray(inputs["bo"], np.float32)
    out = np.zeros((B, T, D), np.float32)
    for b in range(B):
        acc = np.zeros((T, D), np.float32)
        for g in range(4):
            part = np.asarray(res.results[b * 4 + g]["outp"], np.float32)
            acc += part.transpose(1, 0, 2).reshape(T, D)
        out[b] = acc + bo[None, :]
    return out
